# revision 1
# baseline (speedup 1.0000x reference)
"""Trainium2 Bass kernel for GQA causal self-attention (nn_CausalSelfAttention).

Model (hardcoded from the problem spec):
  B=2, T=2048, C=2048, n_head=32, n_kv=8, hs=64
  qkv = x @ w_attn.T + b_attn ; causal GQA attention ; y @ w_proj.T + b_proj

Sharding over 8 cores: core g handles batch b = g//4 and head-group grp = g%4
(8 q-heads, 2 kv-heads per core).  c_attn columns and c_proj rows are split
head-wise; the c_proj partial sums are reduced on the host (the "all-reduce").

Device layout notes:
 - All matmuls contract over the partition dim.  Host pre-transposes x and the
   weight slices so no on-device transposes are needed.
 - Scores are computed K-stationary: S.T tile [tk, tq] = kT.T @ q, so softmax's
   P.T is directly the moving operand of the PV matmul (no P transpose).
 - exp without max-subtraction (scores are ~N(0,1); exp is safe in f32).
 - softmax denominator = ones-row appended to V (row 64 of the PV output);
   normalization multiplies by a [1,tq] reciprocal broadcast to 64 partitions
   via gpsimd partition_broadcast.
 - q rows are stored interleaved ([h0,h4 | h1,h5 | h2,h6 | h3,h7] 64-row
   blocks) so each head's q/k share the same SBUF base partition (0 or 64).
 - heads are processed in pairs (h, h+4): their score matmuls use PE array
   rows 0:63 vs 64:127 (tile_position row groups) and are emitted adjacently
   so the hardware overlaps them; both land in one 2-bank psum tile so a
   single exp covers the pair.
 - block-causal: only tk-tiles <= the tq-tile are computed; in diagonal
   blocks the fully-masked leading columns are skipped in the matmul, exp,
   and PV (psum/pt slots are pre-zeroed so skipped regions stay finite).
 - emission is software-pipelined: projections for token-slice j+1 and
   c_proj for slice j-1 are round-robined between the attention units of
   slice j, keeping the PE busy while exps drain.
"""

import sys
import numpy as np
import ml_dtypes
from contextlib import ExitStack

for _p in ("/opt/trn_rl_repo", "/root/.axon_site/_ro/trn_rl_repo"):
    if _p not in sys.path:
        sys.path.append(_p)

import concourse.mybir as mybir
import concourse.tile as tile
from concourse import bacc
from concourse.bass_utils import run_bass_kernel_spmd

BF16 = mybir.dt.bfloat16
F32 = mybir.dt.float32
NPBF16 = ml_dtypes.bfloat16

B, T, C = 2, 2048, 2048
N_HEAD, N_KV, HS = 32, 8, 64
NE = 2048
N_CORES = 8
HL = 8          # q heads per core
KVL = 2         # kv heads per core
P = 128
TQ = 512        # tq tile (matmul moving width)
NJ = T // TQ    # 4 tq tiles
NT = T // P     # 16 token tiles
KC = C // P     # 16 contraction tiles over channels
QROWS = HL * HS          # 512 local q rows
KROWS = KVL * HS         # 128 local k rows
WCOLS = QROWS + 2 * KROWS  # 768 local w_attn rows

# position-block -> local head: q_sb m-tile mt rows [0:64]=head mt, [64:128]=head mt+4
Q_ORDER = [0, 4, 1, 5, 2, 6, 3, 7]

_CACHE = {}


def _build_program():
    nc = bacc.Bacc("TRN2", target_bir_lowering=False, debug=False)

    xT_d = nc.dram_tensor("xT", [C, T], BF16, kind="ExternalInput")
    wqkvT_d = nc.dram_tensor("wqkvT", [C, WCOLS], BF16, kind="ExternalInput")
    wpT_d = nc.dram_tensor("wpT", [QROWS, C], BF16, kind="ExternalInput")
    bq_d = nc.dram_tensor("bq", [4, P], F32, kind="ExternalInput")
    bk_d = nc.dram_tensor("bk", [1, P], F32, kind="ExternalInput")
    out_d = nc.dram_tensor("out", [T, C], F32, kind="ExternalOutput")

    with tile.TileContext(nc) as tc:
        with ExitStack() as ctx:
            _emit(ctx, tc, nc, xT_d, wqkvT_d, wpT_d, bq_d, bk_d, out_d)
    nc.compile()
    return nc


def _emit(ctx, tc, nc, xT_d, wqkvT_d, wpT_d, bq_d, bk_d, out_d):
    ExpF = mybir.ActivationFunctionType.Exp
    add = mybir.AluOpType.add
    mult = mybir.AluOpType.mult

    persist = ctx.enter_context(tc.tile_pool(name="persist", bufs=1))
    ppa = ctx.enter_context(tc.tile_pool(name="ppa", bufs=2, space="PSUM"))
    pps = ctx.enter_context(tc.tile_pool(name="pps", bufs=2, space="PSUM"))
    ppo = ctx.enter_context(tc.tile_pool(name="ppo", bufs=2, space="PSUM"))
    ptpool = ctx.enter_context(tc.tile_pool(name="pt", bufs=6))
    rcpool = ctx.enter_context(tc.tile_pool(name="rc", bufs=4))
    bcpool = ctx.enter_context(tc.tile_pool(name="bc", bufs=4))
    mkpool = ctx.enter_context(tc.tile_pool(name="mk", bufs=2))
    outpool = ctx.enter_context(tc.tile_pool(name="os", bufs=6))
    obpool = ctx.enter_context(tc.tile_pool(name="ob", bufs=5))

    # ---- persistent SBUF tensors ----
    xT_sb = persist.tile([P, KC * T], BF16, tag="xT")
    wqkv_sb = persist.tile([P, KC * WCOLS], BF16, tag="wqkv")
    wp_sb = persist.tile([P, 4 * C], BF16, tag="wp")
    q_sb = persist.tile([P, 4 * T], BF16, tag="q")
    kT_sb = persist.tile([P, T], BF16, tag="k")
    v_sb = persist.tile([P, NT * 130], BF16, tag="v")
    y_sb = persist.tile([P, 4 * T], BF16, tag="y")
    bq_sb = persist.tile([P, 4], F32, tag="bq")
    bk_sb = persist.tile([P, 1], F32, tag="bk")
    # mask variants for diagonal blocks, doubled for the head-pair layout:
    # maskv[r][x, y] = 1 if (y mod 512)-x-128r >= 0 else 0
    maskv = [persist.tile([P, 2 * TQ], BF16, tag=f"mask{r}", name=f"mask{r}")
             for r in range(4)]

    # ---- input DMAs ----
    # critical first window: weight tile k and its first xT chunk alternate
    for k in range(KC):
        nc.sync.dma_start(wqkv_sb[:, k * WCOLS:(k + 1) * WCOLS],
                          wqkvT_d.ap()[k * P:(k + 1) * P, :])
        nc.sync.dma_start(xT_sb[:, k * T: k * T + TQ],
                          xT_d.ap()[k * P:(k + 1) * P, 0:TQ])
    nc.sync.dma_start(bq_sb[:], bq_d.ap().rearrange("t p -> p t"))
    nc.sync.dma_start(bk_sb[:], bk_d.ap().rearrange("t p -> p t"))
    # remaining xT token chunks (proj(n) starts after chunk n)
    for n in range(1, NJ):
        for k in range(KC):
            nc.sync.dma_start(xT_sb[:, k * T + n * TQ: k * T + (n + 1) * TQ],
                              xT_d.ap()[k * P:(k + 1) * P, n * TQ:(n + 1) * TQ])
    for k in range(4):
        nc.sync.dma_start(wp_sb[:, k * C:(k + 1) * C],
                          wpT_d.ap()[k * P:(k + 1) * P, :])

    # ---- constants ----
    for r in range(4):
        mf = mkpool.tile([P, TQ], F32, tag="mf")
        nc.gpsimd.memset(mf[:], 1.0)
        nc.gpsimd.affine_select(
            out=mf[:], in_=mf[:], compare_op=mybir.AluOpType.is_ge,
            fill=0.0, base=-128 * r, pattern=[[1, TQ]], channel_multiplier=-1)
        nc.scalar.copy(maskv[r][:, 0:TQ], mf[:])
        nc.scalar.copy(maskv[r][:, TQ:2 * TQ], mf[:])
    nc.vector.memset(v_sb[:], 1.0)  # ones columns; data cols overwritten below
    # pre-zero the score psum slots: diagonal blocks are computed at reduced
    # width, so the masked-off region must hold finite values for exp()
    for w in range(2):
        pwarm = pps.tile([P, 2 * TQ], F32, tag="ps", name="pswarm")
        nc.vector.memset(pwarm[:], 0.0)
    for w in range(6):
        ptwarm = ptpool.tile([P, 2 * TQ], BF16, tag="pt", name="ptwarm")
        nc.gpsimd.memset(ptwarm[:], 0.0)

    def xt(k, c0, n):      # xT_sb [c-tile k][:, c0:c0+n] (token cols)
        return xT_sb[:, k * T + c0: k * T + c0 + n]

    def wq(k, mt):         # [128, 128] q-weight tile
        return wqkv_sb[:, k * WCOLS + mt * P: k * WCOLS + (mt + 1) * P]

    def wk(k):
        return wqkv_sb[:, k * WCOLS + QROWS: k * WCOLS + QROWS + P]

    def wv(k):
        return wqkv_sb[:, k * WCOLS + QROWS + P: k * WCOLS + QROWS + 2 * P]

    # ---- work units ----
    # During the startup window (n == 0) the attention PSUM pools are idle,
    # so first-slice projection units borrow their banks for extra overlap.
    def _ppool(pool_sel):
        if pool_sel == 1:
            return pps, "ps"
        if pool_sel == 2:
            return ppo, "po"
        return ppa, "pa"

    def unit_q(n, mt, pool_sel=0):
        def go():
            pool, tg = _ppool(pool_sel)
            ps = pool.tile([P, TQ], F32, tag=tg, name="psq")
            for k in range(KC):
                nc.tensor.matmul(ps[:], wq(k, mt), xt(k, n * TQ, TQ),
                                 start=(k == 0), stop=(k == KC - 1))
            nc.vector.tensor_scalar(
                out=q_sb[:, mt * T + n * TQ: mt * T + (n + 1) * TQ],
                in0=ps[:], scalar1=bq_sb[:, mt:mt + 1], scalar2=None, op0=add)
        return go

    def unit_k(n, pool_sel=0):
        def go():
            pool, tg = _ppool(pool_sel)
            ps = pool.tile([P, TQ], F32, tag=tg, name="psk")
            for k in range(KC):
                nc.tensor.matmul(ps[:], wk(k), xt(k, n * TQ, TQ),
                                 start=(k == 0), stop=(k == KC - 1))
            nc.vector.tensor_scalar(
                out=kT_sb[:, n * TQ:(n + 1) * TQ],
                in0=ps[:], scalar1=0.125, scalar2=bk_sb[:, 0:1],
                op0=mult, op1=add)
        return go

    def unit_v(i, pool_sel=0):
        # v_sb tile i: [0:64]=kv0, 64=ones, [65:129]=kv1, 129=ones
        def go():
            pool, tg = _ppool(pool_sel)
            ps = pool.tile([P, TQ], F32, tag=tg, name="psv")
            for k in range(KC):
                nc.tensor.matmul(ps[:, 0:P], xt(k, i * P, P), wv(k),
                                 start=(k == 0), stop=(k == KC - 1))
            nc.vector.tensor_copy(v_sb[:, i * 130: i * 130 + 64], ps[:, 0:64])
            nc.vector.tensor_copy(v_sb[:, i * 130 + 65: i * 130 + 129],
                                  ps[:, 64:128])
        return go

    def unit_attn(j, hp):
        # processes the head pair (hp, hp+4): same q/y column tile `hp`,
        # head A on partitions 0:64 (kv0), head B on 64:128 (kv1).  Their
        # score matmuls are emitted adjacently so the PE runs them
        # concurrently on disjoint row-groups (tile_position 0 vs 64).
        def go():
            nb = 4 * (j + 1)   # tk tiles in play (block-causal)
            mt = hp
            qcol = mt * T + j * TQ
            po = {}
            po[0] = ppo.tile([65, TQ], F32, tag="po", name="poA")
            po[1] = ppo.tile([65, TQ], F32, tag="po", name="poB")
            for i in range(nb):
                # ps cols [0:512] = head hp (array rows 0:64),
                #         [512:1024] = head hp+4 (array rows 64:128)
                ps = pps.tile([P, 2 * TQ], F32, tag="ps", name="pss")
                # diagonal blocks: cols < 128r are fully masked, skip them
                c0 = max(0, (i - 4 * j)) * P
                for h in (0, 1):
                    rb = 64 * h
                    nc.tensor.matmul(
                        ps[:, h * TQ + c0:(h + 1) * TQ],
                        kT_sb[rb:rb + 64, i * P:(i + 1) * P],
                        q_sb[rb:rb + 64, qcol + c0: qcol + TQ],
                        start=True, stop=True)
                pt = ptpool.tile([P, 2 * TQ], BF16, tag="pt", name="pt")
                nc.scalar.activation(pt[:, c0:2 * TQ], ps[:, c0:2 * TQ], ExpF)
                r = i - 4 * j
                if r >= 0:  # diagonal block: mask both head halves at once
                    nc.vector.tensor_tensor(
                        out=pt[:, c0:2 * TQ], in0=pt[:, c0:2 * TQ],
                        in1=maskv[r][:, c0:2 * TQ], op=mult)
                for h in (0, 1):
                    nc.tensor.matmul(
                        po[h][:, c0:TQ],
                        v_sb[:, i * 130 + 65 * h: i * 130 + 65 * h + 65],
                        pt[:, h * TQ + c0:(h + 1) * TQ],
                        start=(i == 0), stop=(i == nb - 1))
            # normalize: y = po[0:64] * broadcast(1/po[64]); copy PSUM out
            # first so the bank frees for the next head pair.
            for h in (0, 1):
                rb = 64 * h
                ob = obpool.tile([65, TQ], F32, tag="ob", name="ob")
                nc.vector.tensor_copy(ob[:], po[h][:])
                rc = rcpool.tile([1, TQ], F32, tag="rc", name="rc")
                nc.vector.reciprocal(rc[:], ob[64:65, :])
                bc = bcpool.tile([64, TQ], F32, tag="bc", name="bc")
                nc.gpsimd.partition_broadcast(bc[:], rc[:])
                nc.vector.tensor_tensor(
                    out=y_sb[rb:rb + 64, qcol: qcol + TQ],
                    in0=ob[0:64, :], in1=bc[:], op=mult)
        return go

    def unit_cproj(j, ms, ns=range(NJ)):
        def go():
            for n in ns:
                pc = ppa.tile([P, TQ], F32, tag="pa", name="pc")
                for k in range(4):
                    nc.tensor.matmul(
                        pc[:],
                        y_sb[:, k * T + j * TQ + ms * P: k * T + j * TQ + (ms + 1) * P],
                        wp_sb[:, k * C + n * TQ: k * C + (n + 1) * TQ],
                        start=(k == 0), stop=(k == 3))
                os_t = outpool.tile([P, TQ], F32, tag="os", name="os")
                nc.vector.tensor_copy(os_t[:], pc[:])
                nc.sync.dma_start(
                    out_d.ap()[j * TQ + ms * P: j * TQ + (ms + 1) * P,
                               n * TQ:(n + 1) * TQ],
                    os_t[:])
        return go

    def proj_units(n):
        return ([unit_q(n, mt) for mt in range(4)] + [unit_k(n)]
                + [unit_v(i) for i in range(4 * n, 4 * n + 4)])

    def interleave(a, b):
        """Merge unit lists evenly (a paced across b)."""
        out = []
        la, lb = len(a), len(b)
        if la == 0:
            return list(b)
        if lb == 0:
            return list(a)
        ia = ib = 0
        tot = la + lb
        for s in range(tot):
            if ia * lb <= ib * la and ia < la:
                out.append(a[ia]); ia += 1
            elif ib < lb:
                out.append(b[ib]); ib += 1
            else:
                out.append(a[ia]); ia += 1
        return out

    # ---- software-pipelined emission ----
    # P(0) first (spread over all psum pools); then per j: A(j) interleaved
    # with P(j+1) and C(j-1).
    p0 = ([unit_k(0, pool_sel=0)]
          + [unit_q(0, mt, pool_sel=[0, 1, 1, 2][mt]) for mt in range(4)]
          + [unit_v(i, pool_sel=[1, 2, 1, 0][i]) for i in range(4)])
    for u in p0:
        u()
    for j in range(NJ):
        attn = [unit_attn(j, hp) for hp in range(4)]
        filler = []
        if j + 1 < NJ:
            filler += proj_units(j + 1)
        # c_proj work is deferred one extra window where possible so the
        # ACT-bound final windows get more PE filler
        if j == NJ - 1:
            filler += [unit_cproj(jj, ms) for jj in (j - 2, j - 1)
                       for ms in range(4)]
        elif j - 1 >= 1:
            filler += [unit_cproj(j - 2, ms) for ms in range(4)]
        # keep a few filler units after the last attention unit of the
        # window so the PE has work while the final exps drain
        ntail = min(5, len(filler))
        head_f, tail_f = filler[:len(filler) - ntail], filler[len(filler) - ntail:]
        for u in interleave(attn, head_f) + tail_f:
            u()
    for ms in range(4):
        unit_cproj(NJ - 1, ms)()
    # c_proj(0) ran in window 2 via the deferred schedule; nothing left here



def _prep_inputs(x, w_attn, b_attn, w_proj):
    """Host-side shard + transpose + cast for each of the 8 cores."""
    in_maps = []
    for g in range(N_CORES):
        b, grp = divmod(g, 4)
        xT = np.ascontiguousarray(np.asarray(x[b], np.float32).T).astype(NPBF16)

        q_rows = []
        for lh in Q_ORDER:
            gh = HL * grp + lh
            q_rows.extend(range(HS * gh, HS * gh + HS))
        k0 = NE + KROWS * grp
        v0 = NE + N_KV * HS + KROWS * grp
        rows = q_rows + list(range(k0, k0 + KROWS)) + list(range(v0, v0 + KROWS))
        wqkvT = np.ascontiguousarray(w_attn[rows, :].T).astype(NPBF16)

        cols = []
        for lh in Q_ORDER:
            gh = HL * grp + lh
            cols.extend(range(HS * gh, HS * gh + HS))
        wpT = np.ascontiguousarray(w_proj[:, cols].T).astype(NPBF16)

        bq = np.asarray(b_attn[q_rows], np.float32).reshape(4, P)
        bk = (np.asarray(b_attn[k0:k0 + KROWS], np.float32) / 8.0).reshape(1, P)

        in_maps.append({"xT": xT, "wqkvT": wqkvT, "wpT": wpT,
                        "bq": bq, "bk": bk})
    return in_maps


def get_nc():
    if "nc" not in _CACHE:
        _CACHE["nc"] = _build_program()
    return _CACHE["nc"]


def kernel(x, w_attn, b_attn, w_proj, b_proj):
    x = np.asarray(x, np.float32)
    w_attn = np.asarray(w_attn, np.float32)
    b_attn = np.asarray(b_attn, np.float32)
    w_proj = np.asarray(w_proj, np.float32)
    b_proj = np.asarray(b_proj, np.float32)

    nc = get_nc()
    in_maps = _prep_inputs(x, w_attn, b_attn, w_proj)
    res = run_bass_kernel_spmd(nc, in_maps, core_ids=list(range(N_CORES)))

    # host "all-reduce" over the 4 head-group cores per batch + bias folds
    bv = b_attn[NE + N_KV * HS:]                      # [512] v bias
    bv_full = np.repeat(bv.reshape(N_KV, HS), N_HEAD // N_KV, axis=0).reshape(-1)
    delta = bv_full @ w_proj.T + b_proj               # [2048]
    out = np.zeros((B, T, C), np.float32)
    for g in range(N_CORES):
        b = g // 4
        out[b] += res.results[g]["out"]
    out += delta[None, None, :]
    return out



# revision 3
# speedup vs baseline: 1.0226x; 1.0226x over previous
"""Trainium2 Bass kernel for GQA causal self-attention (nn_CausalSelfAttention).

Model (hardcoded from the problem spec):
  B=2, T=2048, C=2048, n_head=32, n_kv=8, hs=64
  qkv = x @ w_attn.T + b_attn ; causal GQA attention ; y @ w_proj.T + b_proj

Sharding over 8 cores: core g handles batch b = g//4 and head-group grp = g%4
(8 q-heads, 2 kv-heads per core).  c_attn columns and c_proj rows are split
head-wise; the c_proj partial sums are reduced on the host (the "all-reduce").

v2 design notes (on top of the v1 baseline):
 - qkv projection in fp8e4m3 DoubleRow with an h+l (high + low residual)
   decomposition: x = xh + xl, 32*w = wh + wl, all fp8, split on the host.
   Three DoubleRow chains per output tile (wh@xh, wh@xl, wl@xh), each
   contracting 2 k-tiles per instruction = 0.75x of the bf16 PE cost at
   ~bf16 accuracy (xl@wl dropped).  The 1/32 weight prescale is undone in
   the psum->sbuf bias-add copy.
 - scores stay bf16, K-stationary (S.T tile [tk, tq]) as in v1.
 - causal masking multiplies only the 128-wide diagonal triangle blocks.
 - PV is "flipped": out y[tq, hs+1] with stationary pt-chunks [tk, 128],
   moving v [tk, 65] (64 dims + ones column -> softmax denominator).
   PE cost 65 per (i-tile, head, tq-subtile) vs 512 in the [hs, tq]
   orientation.  The 8 per-head [128, 65] accumulators of a head share
   one PSUM bank via the per-byte pending-zero protocol: the first matmul
   of a bank starts the group, later slices use start=False +
   skip_group_check (their first write lands on pending-zero bytes and
   overwrites; subsequent writes accumulate).
 - softmax normalization: gpsimd normalize_recip (Pool engine) divides
   y[tq, hs] rows by the denominator column, writing bf16 directly.
 - y is transposed back to [hs, tq] with PE transposes (4 transposes of
   one head pair packed into one psum bank) for the c_proj matmul.
"""

import sys
import numpy as np
import ml_dtypes
from contextlib import ExitStack

for _p in ("/opt/trn_rl_repo", "/root/.axon_site/_ro/trn_rl_repo"):
    if _p not in sys.path:
        sys.path.append(_p)

import concourse.mybir as mybir
import concourse.tile as tile
from concourse import bacc
from concourse.bass_utils import run_bass_kernel_spmd
from concourse.masks import make_identity

BF16 = mybir.dt.bfloat16
F32 = mybir.dt.float32
FP8 = mybir.dt.float8e4
NPBF16 = ml_dtypes.bfloat16
NPFP8 = ml_dtypes.float8_e4m3
DR = mybir.MatmulPerfMode.DoubleRow

B, T, C = 2, 2048, 2048
N_HEAD, N_KV, HS = 32, 8, 64
NE = 2048
N_CORES = 8
HL = 8          # q heads per core
KVL = 2         # kv heads per core
P = 128
TQ = 512        # tq window (matmul moving width)
NJ = T // TQ    # 4 tq windows
NT = T // P     # 16 token tiles
KC = C // P     # 16 contraction tiles over channels
QROWS = HL * HS          # 512 local q rows
KROWS = KVL * HS         # 128 local k rows
WCOLS = QROWS + 2 * KROWS  # 768 local w_attn rows
WSCALE = 32.0   # host prescale on w_attn so fp8 residuals stay in range

# position-block -> local head: q_sb m-tile mt rows [0:64]=head mt, [64:128]=head mt+4
Q_ORDER = [0, 4, 1, 5, 2, 6, 3, 7]

_CACHE = {}

# scheduling knobs (overridable before get_nc())
BUDGETS = [23200, 15400, 25600, 20500]
TAIL_STEPS = 1
TAIL_FIRST = False
PV_LAG = 3
PT_BUFS = 5
MASK_ON_POOL = False
YS_BUFS = 6
OS_BUFS = 3
CPROJ_AT = {2: [(0, ms) for ms in range(4)],
            3: [(jj, ms) for jj in (1, 2) for ms in range(4)]}


def _build_program():
    nc = bacc.Bacc("TRN2", target_bir_lowering=False, debug=False)

    xhl_d = nc.dram_tensor("xhl", [C, 2 * T], FP8, kind="ExternalInput")
    whl_d = nc.dram_tensor("whl", [C, 2 * WCOLS], FP8, kind="ExternalInput")
    wphl_d = nc.dram_tensor("wphl", [QROWS, 2 * C], FP8, kind="ExternalInput")
    bq_d = nc.dram_tensor("bq", [4, P], F32, kind="ExternalInput")
    bk_d = nc.dram_tensor("bk", [1, P], F32, kind="ExternalInput")
    out_d = nc.dram_tensor("out", [T, C], F32, kind="ExternalOutput")

    with tile.TileContext(nc) as tc:
        with ExitStack() as ctx:
            _emit(ctx, tc, nc, xhl_d, whl_d, wphl_d, bq_d, bk_d, out_d)
    nc.compile()
    return nc


def _emit(ctx, tc, nc, xhl_d, whl_d, wphl_d, bq_d, bk_d, out_d):
    MASK_ENG = nc.gpsimd if MASK_ON_POOL else nc.vector
    ExpF = mybir.ActivationFunctionType.Exp
    add = mybir.AluOpType.add
    mult = mybir.AluOpType.mult

    persist = ctx.enter_context(tc.tile_pool(name="persist", bufs=1))
    pps = ctx.enter_context(tc.tile_pool(name="pps", bufs=2, space="PSUM"))
    ppv = ctx.enter_context(tc.tile_pool(name="ppv", bufs=2, space="PSUM"))
    ppm = ctx.enter_context(tc.tile_pool(name="ppm", bufs=2, space="PSUM"))
    ptpool = ctx.enter_context(tc.tile_pool(name="pt", bufs=PT_BUFS))
    yspool = ctx.enter_context(tc.tile_pool(name="ys", bufs=YS_BUFS))
    ynpool = ctx.enter_context(tc.tile_pool(name="yn", bufs=4))
    outpool = ctx.enter_context(tc.tile_pool(name="os", bufs=OS_BUFS))

    # ---- persistent SBUF tensors ----
    # plane dim: 0 = h (fp8 high), 1 = l (fp8 residual)
    xhl_sb = persist.tile([P, KC, 2, T], FP8, tag="xhl")
    whl_sb = persist.tile([P, KC, 2, WCOLS], FP8, tag="whl")
    wphl_sb = persist.tile([P, 4, 2, C], FP8, tag="wphl")
    q_sb = persist.tile([P, 4 * T], BF16, tag="q")
    kT_sb = persist.tile([P, T], BF16, tag="k")
    v_sb = persist.tile([P, NT * 130], BF16, tag="v")
    yhl_sb = persist.tile([P, 2, 4, T], FP8, tag="yhl")
    bq_sb = persist.tile([P, 4], F32, tag="bq")
    bk_sb = persist.tile([P, 1], F32, tag="bk")
    ident = persist.tile([P, P], BF16, tag="ident")
    # triangle mask for diagonal blocks: trimask[x, y] = 1 if y >= x else 0
    trimask = persist.tile([P, P], BF16, tag="trimask")

    # ---- input DMAs ----
    # One DMA per (k-tile [, token chunk]): every DMA serializes ~630 ns on
    # the shared HWDGE device, so fewer/bigger transfers pace the startup.
    # Emission order matches consumption: w, x chunk 0, chunk 1, wp (needed
    # by cproj(0) during window 1), chunks 2-3.
    def wsrc(kq, pl):
        # 4 k-tiles of one w plane: [128, 4, WCOLS]
        return whl_d.ap()[4 * kq * P:(4 * kq + 4) * P, :].rearrange(
            "(kk p) (two w) -> p kk two w", p=P, two=2)[:, :, pl, :]

    def xsrc(kq, pl, n):
        return xhl_d.ap()[4 * kq * P:(4 * kq + 4) * P, :].rearrange(
            "(kk p) (two t) -> p kk two t", p=P, two=2)[:, :, pl, n * TQ:(n + 1) * TQ]

    def wdst(kq, pl):
        return whl_sb[:, 4 * kq:4 * kq + 4, pl, :]

    def xdst(kq, pl, n):
        return xhl_sb[:, 4 * kq:4 * kq + 4, pl, n * TQ:(n + 1) * TQ]

    # order matches chain consumption: (wh,xh) mains, then xl, then wl
    for kq in range(4):
        nc.sync.dma_start(wdst(kq, 0), wsrc(kq, 0))
        nc.sync.dma_start(xdst(kq, 0, 0), xsrc(kq, 0, 0))
    nc.sync.dma_start(bq_sb[:], bq_d.ap().rearrange("t p -> p t"))
    nc.sync.dma_start(bk_sb[:], bk_d.ap().rearrange("t p -> p t"))
    for kq in range(4):
        nc.sync.dma_start(xdst(kq, 1, 0), xsrc(kq, 1, 0))
    for kq in range(4):
        nc.sync.dma_start(wdst(kq, 1), wsrc(kq, 1))
    for kq in range(4):
        nc.sync.dma_start(xdst(kq, 0, 1), xsrc(kq, 0, 1))
        nc.sync.dma_start(xdst(kq, 1, 1), xsrc(kq, 1, 1))
    for k in range(4):
        nc.sync.dma_start(wphl_sb[:, k, :, :],
                          wphl_d.ap()[k * P:(k + 1) * P, :].rearrange(
                              "p (two c) -> p two c", two=2))
    for n in range(2, NJ):
        for kq in range(4):
            nc.sync.dma_start(xdst(kq, 0, n), xsrc(kq, 0, n))
            nc.sync.dma_start(xdst(kq, 1, n), xsrc(kq, 1, n))

    # ---- constants ----
    make_identity(nc, ident[:])
    mkf = ynpool.tile([P, P], F32, tag="yn", name="mkf")
    nc.gpsimd.memset(mkf[:], 1.0)
    nc.gpsimd.affine_select(
        out=mkf[:], in_=mkf[:], compare_op=mybir.AluOpType.is_ge,
        fill=0.0, base=0, pattern=[[1, P]], channel_multiplier=-1)
    nc.scalar.copy(trimask[:], mkf[:])
    nc.vector.memset(v_sb[:], 1.0)  # ones columns; data cols overwritten below
    # pre-zero the score psum tiles: exp may read never-written columns in
    # diagonal windows (they are masked later), which must be finite
    for w in range(2):
        pwarm = pps.tile([P, 2 * TQ], F32, tag="ps", name="pswarm")
        nc.vector.memset(pwarm[:], 0.0)

    # ---- qkv projection units (fp8 h+l DoubleRow) ----
    def qkv_matmuls(ps_ap, wcol0, wcols, tcol0, tcols, swap=False):
        """Three DoubleRow chains -- (wh,xh), (wh,xl), (wl,xh) -- each
        contracting 2 k-tiles per instruction.  swap: x stationary."""
        for ci, (wp_, xp_) in enumerate([(0, 0), (0, 1), (1, 0)]):
            for kp in range(KC // 2):
                wap = whl_sb[:, 2 * kp:2 * kp + 2, wp_, wcol0:wcol0 + wcols]
                xap = xhl_sb[:, 2 * kp:2 * kp + 2, xp_, tcol0:tcol0 + tcols]
                lhsT, rhs = (xap, wap) if swap else (wap, xap)
                nc.tensor.matmul(ps_ap, lhsT, rhs,
                                 start=(ci == 0 and kp == 0),
                                 stop=(ci == 2 and kp == KC // 2 - 1),
                                 perf_mode=DR)

    def unit_q(n, mt):
        def go():
            ps = ppm.tile([P, TQ], F32, tag="pm", name="psq")
            qkv_matmuls(ps[:], mt * P, P, n * TQ, TQ)
            nc.vector.tensor_scalar(
                out=q_sb[:, mt * T + n * TQ: mt * T + (n + 1) * TQ],
                in0=ps[:], scalar1=1.0 / WSCALE, scalar2=bq_sb[:, mt:mt + 1],
                op0=mult, op1=add)
        return go

    def unit_k(n):
        def go():
            ps = ppm.tile([P, TQ], F32, tag="pm", name="psk")
            qkv_matmuls(ps[:], QROWS, P, n * TQ, TQ)
            nc.vector.tensor_scalar(
                out=kT_sb[:, n * TQ:(n + 1) * TQ],
                in0=ps[:], scalar1=0.125 / WSCALE, scalar2=bk_sb[:, 0:1],
                op0=mult, op1=add)
        return go

    def unit_v(i):
        # v_sb tile i: [0:64]=kv0, 64=ones, [65:129]=kv1, 129=ones
        def go():
            ps = ppm.tile([P, TQ], F32, tag="pm", name="psv")
            qkv_matmuls(ps[:, 0:P], QROWS + P, P, i * P, P, swap=True)
            nc.vector.tensor_scalar(
                out=v_sb[:, i * 130: i * 130 + 64], in0=ps[:, 0:64],
                scalar1=1.0 / WSCALE, scalar2=None, op0=mult)
            nc.vector.tensor_scalar(
                out=v_sb[:, i * 130 + 65: i * 130 + 129], in0=ps[:, 64:128],
                scalar1=1.0 / WSCALE, scalar2=None, op0=mult)
        return go

    # ---- attention units ----
    def unit_attn(j, hp):
        # head pair (hp, hp+4): q/y column tile `hp`, head A on partitions
        # 0:64 of q/kT (kv0), head B on 64:128 (kv1).
        # Main generator: one yield per i-tile "slot"; PV trails scores by
        # 2 i-tiles so each PV matmul's exp finished a full slot earlier
        # (the PE is in-order -- a waiting matmul blocks the queue).
        # The normalize/transpose tail is returned as a separate generator
        # that the scheduler interleaves with the NEXT unit's slots.
        nb = 4 * (j + 1)   # tk tiles in play (block-causal)
        qcol = hp * T + j * TQ
        pv = [None, None]
        pv_first = [True, True]
        pts = {}
        ys = [None, None]

        def emit_scores_exp(i):
            r = i - 4 * j
            c0 = max(0, r) * P
            ps = pps.tile([P, 2 * TQ], F32, tag="ps", name="pss")
            for h in (0, 1):
                rb = 64 * h
                nc.tensor.matmul(
                    ps[:, TQ * h + c0: TQ * (h + 1)],
                    kT_sb[rb:rb + 64, i * P:(i + 1) * P],
                    q_sb[rb:rb + 64, qcol + c0: qcol + TQ],
                    start=True, stop=True)
            pt = ptpool.tile([P, 2 * TQ], BF16, tag="pt", name="pt")
            pts[i] = pt
            if c0 >= 2 * P:
                # skip the head-B hole [TQ, TQ+c0) -- two exps are cheaper
                # than the wasted columns once c0 >= 256
                nc.scalar.activation(pt[:, c0:TQ], ps[:, c0:TQ], ExpF)
                nc.scalar.activation(pt[:, TQ + c0:2 * TQ], ps[:, TQ + c0:2 * TQ], ExpF)
            else:
                nc.scalar.activation(pt[:, c0:2 * TQ], ps[:, c0:2 * TQ], ExpF)
            if r >= 0:
                for h in (0, 1):
                    tri = pt[:, TQ * h + c0: TQ * h + c0 + P]
                    MASK_ENG.tensor_tensor(out=tri, in0=tri, in1=trimask[:], op=mult)

        def emit_pv(i):
            pt = pts.pop(i)
            r = i - 4 * j
            for h in (0, 1):
                for s in range(max(0, r), 4):
                    first = pv_first[h]
                    pv_first[h] = False
                    nc.tensor.matmul(
                        pv[h][:, 65 * s: 65 * s + 65],
                        pt[:, TQ * h + P * s: TQ * h + P * (s + 1)],
                        v_sb[:, i * 130 + 65 * h: i * 130 + 65 * h + 65],
                        start=first, stop=(i == nb - 1 and s == 3),
                        skip_group_check=not first)

        def main_gen():
            pv[0] = ppv.tile([P, 4 * 65], F32, tag="pv", name="pvA")
            pv[1] = ppv.tile([P, 4 * 65], F32, tag="pv", name="pvB")
            for i in range(nb):
                emit_scores_exp(i)
                if i >= PV_LAG:
                    emit_pv(i - PV_LAG)
                yield 650
            for i in range(max(0, nb - PV_LAG), nb - 1):
                emit_pv(i)
                yield 350
            emit_pv(nb - 1)
            # drain the PV psum banks so the next unit can claim them
            for h in (0, 1):
                ys[h] = yspool.tile([P, 4 * 65], F32, tag="ys", name=f"ys{h}")
                nc.vector.tensor_copy(ys[h][:], pv[h][:])
            yield 350

        def tail_gen():
            # normalize into bf16 (Pool), transpose in bf16 (PE), then h/l
            # fp8 split during the psum->sbuf copies (DVE).  fp8 values are
            # exact in bf16, so yh+yl reproduces the bf16 y exactly.
            for s in range(4):
                for h in (0, 1):
                    o = P * s + 64 * h + hp * TQ
                    nc.gpsimd.normalize_recip(
                        ynf[:, o: o + 64],
                        ys[h][:, 65 * s: 65 * s + 64],
                        ys[h][:, 65 * s + 64: 65 * s + 65])
                yield 0
            yield 0
            ptr = ppm.tile([P, TQ], BF16, tag="pm", name="ptr")
            for s in range(4):
                nc.tensor.matmul(
                    ptr[:, P * s: P * (s + 1)],
                    ynf[:, P * s + hp * TQ: P * (s + 1) + hp * TQ],
                    ident[:], is_transpose=True,
                    start=(s == 0), stop=(s == 3), skip_group_check=(s != 0))
            yh_dst = yhl_sb[:, 0, hp, j * TQ: (j + 1) * TQ]
            nc.vector.tensor_copy(yh_dst, ptr[:])
            nc.vector.tensor_tensor(
                out=yhl_sb[:, 1, hp, j * TQ: (j + 1) * TQ],
                in0=ptr[:], in1=yh_dst, op=mybir.AluOpType.subtract)
        return main_gen(), tail_gen

    def unit_cproj(j, ms, split_dma=False):
        def gen():
            os_t = outpool.tile([P, 4 * TQ], F32, tag="os", name="os")
            for n in range(NJ):
                pc = ppm.tile([P, TQ], F32, tag="pm", name="pc")
                for ci, (yp, wpp) in enumerate([(0, 0), (0, 1), (1, 0)]):
                    for kp in (0, 1):
                        nc.tensor.matmul(
                            pc[:],
                            yhl_sb[:, yp, 2 * kp:2 * kp + 2,
                                   j * TQ + ms * P: j * TQ + (ms + 1) * P],
                            wphl_sb[:, 2 * kp:2 * kp + 2, wpp,
                                    n * TQ:(n + 1) * TQ],
                            start=(ci == 0 and kp == 0),
                            stop=(ci == 2 and kp == 1), perf_mode=DR)
                    if ci == 1:
                        yield 330
                nc.vector.tensor_scalar(
                    out=os_t[:, n * TQ:(n + 1) * TQ], in0=pc[:],
                    scalar1=1.0 / WSCALE, scalar2=None, op0=mult)
                if split_dma:
                    nc.sync.dma_start(
                        out_d.ap()[j * TQ + ms * P: j * TQ + (ms + 1) * P,
                                   n * TQ:(n + 1) * TQ],
                        os_t[:, n * TQ:(n + 1) * TQ])
                yield 430
            if not split_dma:
                nc.sync.dma_start(
                    out_d.ap()[j * TQ + ms * P: j * TQ + (ms + 1) * P, :], os_t[:])
        return gen()

    def gen_q(n, mt):
        def gen():
            ps = ppm.tile([P, TQ], F32, tag="pm", name="psq")
            for seg in _qkv_segs(ps[:], mt * P, P, n * TQ, TQ):
                yield seg
            nc.vector.tensor_scalar(
                out=q_sb[:, mt * T + n * TQ: mt * T + (n + 1) * TQ],
                in0=ps[:], scalar1=1.0 / WSCALE, scalar2=bq_sb[:, mt:mt + 1],
                op0=mult, op1=add)
        return gen()

    def gen_k(n):
        def gen():
            ps = ppm.tile([P, TQ], F32, tag="pm", name="psk")
            for seg in _qkv_segs(ps[:], QROWS, P, n * TQ, TQ):
                yield seg
            nc.vector.tensor_scalar(
                out=kT_sb[:, n * TQ:(n + 1) * TQ],
                in0=ps[:], scalar1=0.125 / WSCALE, scalar2=bk_sb[:, 0:1],
                op0=mult, op1=add)
        return gen()

    def _qkv_segs(ps_ap, wcol0, wcols, tcol0, tcols, swap=False):
        emitted = 0
        for ci, (wp_, xp_) in enumerate([(0, 0), (0, 1), (1, 0)]):
            for kp in range(KC // 2):
                wap = whl_sb[:, 2 * kp:2 * kp + 2, wp_, wcol0:wcol0 + wcols]
                xap = xhl_sb[:, 2 * kp:2 * kp + 2, xp_, tcol0:tcol0 + tcols]
                lhsT, rhs = (xap, wap) if swap else (wap, xap)
                nc.tensor.matmul(ps_ap, lhsT, rhs,
                                 start=(ci == 0 and kp == 0),
                                 stop=(ci == 2 and kp == KC // 2 - 1),
                                 perf_mode=DR)
                emitted += 1
                if emitted % 6 == 0 and emitted < 24:
                    yield 640

    def gen_v(i):
        # v_sb tile i: [0:64]=kv0, 64=ones, [65:129]=kv1, 129=ones
        def gen():
            ps = ppm.tile([P, TQ], F32, tag="pm", name="psv")
            for seg in _qkv_segs(ps[:, 0:P], QROWS + P, P, i * P, P, swap=True):
                pass  # 27 ns per matmul; no need to split
            nc.vector.tensor_scalar(
                out=v_sb[:, i * 130: i * 130 + 64], in0=ps[:, 0:64],
                scalar1=1.0 / WSCALE, scalar2=None, op0=mult)
            nc.vector.tensor_scalar(
                out=v_sb[:, i * 130 + 65: i * 130 + 129], in0=ps[:, 64:128],
                scalar1=1.0 / WSCALE, scalar2=None, op0=mult)
            yield 650
        return gen()

    def proj_gens(n):
        return ([gen_q(n, mt) for mt in range(4)] + [gen_k(n)]
                + [gen_v(i) for i in range(4 * n, 4 * n + 4)])

    # yn staging buffers for normalize->transpose, one [P, TQ] region per pair
    ynf = persist.tile([P, 4 * TQ], BF16, tag="ynf")

    class FillerQueue:
        """Sequential queue of generator units; pulls ~budget ns of PE
        segments at a time."""

        def __init__(self):
            self.gens = []
            self.cur = None
            self.done = 0

        def add(self, gens):
            self.gens.extend(gens)

        def push_front(self, gen):
            self.gens.insert(0, gen)

        def pull(self, budget):
            got = 0
            while got < budget:
                if self.cur is None:
                    if not self.gens:
                        return got
                    self.cur = self.gens.pop(0)
                try:
                    got += next(self.cur)
                except StopIteration:
                    self.cur = None
                    self.done += 1
            return got

        def drain(self):
            while self.pull(1 << 30) > 0:
                pass

    # ---- software-pipelined emission ----
    # Three filler queues by priority: fqd (this window's deferred q m-tiles
    # 1-3 -- needed by attn units 1-3 of the SAME window), fqp (next window's
    # critical projections: q m-tile 0, k, v), fqf (cproj -- fully flexible,
    # carried across windows to feed the ACT-bound late windows).
    fqd = FillerQueue()
    fqp = FillerQueue()
    fqf = FillerQueue()

    def pull(budget):
        got = fqd.pull(budget)
        if got < budget:
            got += fqp.pull(budget - got)
        if got < budget:
            fqf.pull(budget - got)

    def step(g):
        if g is None:
            return None
        try:
            next(g)
            return g
        except StopIteration:
            return None

    # prologue: everything attn(0, hp=0) touches must be fully emitted
    # before its PV reads are emitted (emission order = dependency order)
    fqp.add([gen_q(0, 0), gen_k(0)] + [gen_v(i) for i in range(4)])
    fqp.drain()
    pending_tail = None
    deferred_q = {j: [gen_q(j, mt) for mt in (1, 2, 3)] for j in range(NJ)}
    for j in range(NJ):
        fqd.add(deferred_q[j])
        if j + 1 < NJ:
            fqp.add([gen_q(j + 1, 0), gen_k(j + 1)]
                    + [gen_v(i) for i in range(4 * (j + 1), 4 * (j + 1) + 4)])
        nslots = 4 * (4 * (j + 1) + 3)
        budget = BUDGETS[j] / nslots
        done0 = fqd.done
        for hp in range(4):
            if hp == 1:
                # attn(j-1, 3)'s tail (stepped during unit 0's slots) has
                # fully emitted its yhl write by now; cproj may follow it
                for (jj, ms) in CPROJ_AT.get(j, []):
                    fqf.add([unit_cproj(jj, ms)])
            if hp >= 1:
                # attn(j, hp) needs q(j, hp): force any unemitted remainder
                while (fqd.done - done0 < hp
                       and (fqd.cur is not None or fqd.gens)):
                    fqd.pull(700)
            main, tail = unit_attn(j, hp)
            for hint in main:
                pull(budget)
                for _ in range(TAIL_STEPS):
                    pending_tail = step(pending_tail)
            pending_tail = tail()
        fqp.drain()
    while pending_tail is not None:
        pending_tail = step(pending_tail)
        pull(500)
    fqf.add([unit_cproj(3, ms, split_dma=True) for ms in range(4)])
    fqf.drain()


def _prep_inputs(x, w_attn, b_attn, w_proj):
    """Host-side shard + transpose + fp8 h+l split for each of the 8 cores."""
    in_maps = []
    xhl = {}
    for b in range(B):
        xT = np.ascontiguousarray(np.asarray(x[b], np.float32).T)
        xh = xT.astype(NPFP8)
        xl = (xT - xh.astype(np.float32)).astype(NPFP8)
        xhl[b] = np.concatenate([xh, xl], axis=1)  # [C, 2T]
    for g in range(N_CORES):
        b, grp = divmod(g, 4)

        q_rows = []
        for lh in Q_ORDER:
            gh = HL * grp + lh
            q_rows.extend(range(HS * gh, HS * gh + HS))
        k0 = NE + KROWS * grp
        v0 = NE + N_KV * HS + KROWS * grp
        rows = q_rows + list(range(k0, k0 + KROWS)) + list(range(v0, v0 + KROWS))
        wqkvT = np.ascontiguousarray(w_attn[rows, :].T) * WSCALE
        wh = wqkvT.astype(NPFP8)
        wl = (wqkvT - wh.astype(np.float32)).astype(NPFP8)
        whl = np.concatenate([wh, wl], axis=1)  # [C, 2*WCOLS]

        cols = []
        for lh in Q_ORDER:
            gh = HL * grp + lh
            cols.extend(range(HS * gh, HS * gh + HS))
        wpT = np.ascontiguousarray(w_proj[:, cols].T) * WSCALE
        wph = wpT.astype(NPFP8)
        wpl = (wpT - wph.astype(np.float32)).astype(NPFP8)
        wphl = np.concatenate([wph, wpl], axis=1)  # [QROWS, 2C]

        bq = np.asarray(b_attn[q_rows], np.float32).reshape(4, P)
        bk = (np.asarray(b_attn[k0:k0 + KROWS], np.float32) / 8.0).reshape(1, P)

        in_maps.append({"xhl": xhl[b], "whl": whl, "wphl": wphl,
                        "bq": bq, "bk": bk})
    return in_maps


def get_nc():
    if "nc" not in _CACHE:
        _CACHE["nc"] = _build_program()
    return _CACHE["nc"]


def kernel(x, w_attn, b_attn, w_proj, b_proj):
    x = np.asarray(x, np.float32)
    w_attn = np.asarray(w_attn, np.float32)
    b_attn = np.asarray(b_attn, np.float32)
    w_proj = np.asarray(w_proj, np.float32)
    b_proj = np.asarray(b_proj, np.float32)

    nc = get_nc()
    in_maps = _prep_inputs(x, w_attn, b_attn, w_proj)
    res = run_bass_kernel_spmd(nc, in_maps, core_ids=list(range(N_CORES)))

    # host "all-reduce" over the 4 head-group cores per batch + bias folds
    bv = b_attn[NE + N_KV * HS:]                      # [512] v bias
    bv_full = np.repeat(bv.reshape(N_KV, HS), N_HEAD // N_KV, axis=0).reshape(-1)
    delta = bv_full @ w_proj.T + b_proj               # [2048]
    out = np.zeros((B, T, C), np.float32)
    for g in range(N_CORES):
        b = g // 4
        out[b] += res.results[g]["out"]
    out += delta[None, None, :]
    return out


# revision 4
# speedup vs baseline: 1.0300x; 1.0073x over previous
"""Trainium2 Bass kernel for GQA causal self-attention (nn_CausalSelfAttention).

Model (hardcoded from the problem spec):
  B=2, T=2048, C=2048, n_head=32, n_kv=8, hs=64
  qkv = x @ w_attn.T + b_attn ; causal GQA attention ; y @ w_proj.T + b_proj

Sharding over 8 cores: core g handles batch b = g//4 and head-group grp = g%4
(8 q-heads, 2 kv-heads per core).  c_attn columns and c_proj rows are split
head-wise; the c_proj partial sums are reduced on the host (the "all-reduce").

v2 design notes (on top of the v1 baseline):
 - qkv projection in fp8e4m3 DoubleRow with an h+l (high + low residual)
   decomposition: x = xh + xl, 32*w = wh + wl, all fp8, split on the host.
   Three DoubleRow chains per output tile (wh@xh, wh@xl, wl@xh), each
   contracting 2 k-tiles per instruction = 0.75x of the bf16 PE cost at
   ~bf16 accuracy (xl@wl dropped).  The 1/32 weight prescale is undone in
   the psum->sbuf bias-add copy.
 - scores stay bf16, K-stationary (S.T tile [tk, tq]) as in v1.
 - causal masking multiplies only the 128-wide diagonal triangle blocks.
 - PV is "flipped": out y[tq, hs+1] with stationary pt-chunks [tk, 128],
   moving v [tk, 65] (64 dims + ones column -> softmax denominator).
   PE cost 65 per (i-tile, head, tq-subtile) vs 512 in the [hs, tq]
   orientation.  The 8 per-head [128, 65] accumulators of a head share
   one PSUM bank via the per-byte pending-zero protocol: the first matmul
   of a bank starts the group, later slices use start=False +
   skip_group_check (their first write lands on pending-zero bytes and
   overwrites; subsequent writes accumulate).
 - softmax normalization: gpsimd normalize_recip (Pool engine) divides
   y[tq, hs] rows by the denominator column, writing bf16 directly.
 - y is transposed back to [hs, tq] with PE transposes (4 transposes of
   one head pair packed into one psum bank) for the c_proj matmul.
"""

import sys
import numpy as np
import ml_dtypes
from contextlib import ExitStack

for _p in ("/opt/trn_rl_repo", "/root/.axon_site/_ro/trn_rl_repo"):
    if _p not in sys.path:
        sys.path.append(_p)

import concourse.mybir as mybir
import concourse.tile as tile
from concourse import bacc
from concourse.bass_utils import run_bass_kernel_spmd
from concourse.masks import make_identity

BF16 = mybir.dt.bfloat16
F32 = mybir.dt.float32
FP8 = mybir.dt.float8e4
NPBF16 = ml_dtypes.bfloat16
NPFP8 = ml_dtypes.float8_e4m3
DR = mybir.MatmulPerfMode.DoubleRow

B, T, C = 2, 2048, 2048
N_HEAD, N_KV, HS = 32, 8, 64
NE = 2048
N_CORES = 8
HL = 8          # q heads per core
KVL = 2         # kv heads per core
P = 128
TQ = 512        # tq window (matmul moving width)
NJ = T // TQ    # 4 tq windows
NT = T // P     # 16 token tiles
KC = C // P     # 16 contraction tiles over channels
QROWS = HL * HS          # 512 local q rows
KROWS = KVL * HS         # 128 local k rows
WCOLS = QROWS + 2 * KROWS  # 768 local w_attn rows
WSCALE = 32.0   # host prescale on w_attn so fp8 residuals stay in range

# position-block -> local head: q_sb m-tile mt rows [0:64]=head mt, [64:128]=head mt+4
Q_ORDER = [0, 4, 1, 5, 2, 6, 3, 7]

_CACHE = {}

# scheduling knobs (overridable before get_nc())
BUDGETS = [23200, 15400, 25600, 20500]
TAIL_STEPS = 1
TAIL_FIRST = False
PV_LAG = 3
PT_BUFS = 5
MASK_ON_POOL = False
YS_BUFS = 6
OS_BUFS = 3
CPROJ_AT = {2: [(0, ms) for ms in range(4)],
            3: [(jj, ms) for jj in (1, 2) for ms in range(4)]}


def _build_program():
    nc = bacc.Bacc("TRN2", target_bir_lowering=False, debug=False)

    xhl_d = nc.dram_tensor("xhl", [C, 2 * T], FP8, kind="ExternalInput")
    whl_d = nc.dram_tensor("whl", [C, 2 * WCOLS], FP8, kind="ExternalInput")
    wphl_d = nc.dram_tensor("wphl", [QROWS, 2 * C], FP8, kind="ExternalInput")
    bq_d = nc.dram_tensor("bq", [4, P], F32, kind="ExternalInput")
    bk_d = nc.dram_tensor("bk", [1, P], F32, kind="ExternalInput")
    out_d = nc.dram_tensor("out", [T, C], F32, kind="ExternalOutput")

    with tile.TileContext(nc) as tc:
        with ExitStack() as ctx:
            _emit(ctx, tc, nc, xhl_d, whl_d, wphl_d, bq_d, bk_d, out_d)
    nc.compile()
    return nc


def _emit(ctx, tc, nc, xhl_d, whl_d, wphl_d, bq_d, bk_d, out_d):
    MASK_ENG = nc.gpsimd if MASK_ON_POOL else nc.vector
    ExpF = mybir.ActivationFunctionType.Exp
    add = mybir.AluOpType.add
    mult = mybir.AluOpType.mult

    persist = ctx.enter_context(tc.tile_pool(name="persist", bufs=1))
    pps = ctx.enter_context(tc.tile_pool(name="pps", bufs=2, space="PSUM"))
    ppv = ctx.enter_context(tc.tile_pool(name="ppv", bufs=2, space="PSUM"))
    ppm = ctx.enter_context(tc.tile_pool(name="ppm", bufs=2, space="PSUM"))
    ptpool = ctx.enter_context(tc.tile_pool(name="pt", bufs=PT_BUFS))
    yspool = ctx.enter_context(tc.tile_pool(name="ys", bufs=YS_BUFS))
    ynpool = ctx.enter_context(tc.tile_pool(name="yn", bufs=4))
    outpool = ctx.enter_context(tc.tile_pool(name="os", bufs=OS_BUFS))

    # ---- persistent SBUF tensors ----
    # plane dim: 0 = h (fp8 high), 1 = l (fp8 residual)
    xhl_sb = persist.tile([P, KC, 2, T], FP8, tag="xhl")
    whl_sb = persist.tile([P, KC, 2, WCOLS], FP8, tag="whl")
    wphl_sb = persist.tile([P, 4, 2, C], FP8, tag="wphl")
    q_sb = persist.tile([P, 4 * T], BF16, tag="q")
    kT_sb = persist.tile([P, T], BF16, tag="k")
    v_sb = persist.tile([P, NT * 130], BF16, tag="v")
    yhl_sb = persist.tile([P, 2, 4, T], FP8, tag="yhl")
    bq_sb = persist.tile([P, 4], F32, tag="bq")
    bk_sb = persist.tile([P, 1], F32, tag="bk")
    ident = persist.tile([P, P], BF16, tag="ident")
    # triangle mask for diagonal blocks: trimask[x, y] = 1 if y >= x else 0
    trimask = persist.tile([P, P], BF16, tag="trimask")

    # ---- input DMAs ----
    # One DMA per (k-tile [, token chunk]): every DMA serializes ~630 ns on
    # the shared HWDGE device, so fewer/bigger transfers pace the startup.
    # Emission order matches consumption: w, x chunk 0, chunk 1, wp (needed
    # by cproj(0) during window 1), chunks 2-3.
    def wsrc(kq, pl):
        # 4 k-tiles of one w plane: [128, 4, WCOLS]
        return whl_d.ap()[4 * kq * P:(4 * kq + 4) * P, :].rearrange(
            "(kk p) (two w) -> p kk two w", p=P, two=2)[:, :, pl, :]

    def xsrc(kq, pl, n):
        return xhl_d.ap()[4 * kq * P:(4 * kq + 4) * P, :].rearrange(
            "(kk p) (two t) -> p kk two t", p=P, two=2)[:, :, pl, n * TQ:(n + 1) * TQ]

    def wdst(kq, pl):
        return whl_sb[:, 4 * kq:4 * kq + 4, pl, :]

    def xdst(kq, pl, n):
        return xhl_sb[:, 4 * kq:4 * kq + 4, pl, n * TQ:(n + 1) * TQ]

    # order matches chain consumption: (wh,xh) mains, then xl, then wl
    for kq in range(4):
        nc.sync.dma_start(wdst(kq, 0), wsrc(kq, 0))
        nc.sync.dma_start(xdst(kq, 0, 0), xsrc(kq, 0, 0))
    nc.sync.dma_start(bq_sb[:], bq_d.ap().rearrange("t p -> p t"))
    nc.sync.dma_start(bk_sb[:], bk_d.ap().rearrange("t p -> p t"))
    for kq in range(4):
        nc.sync.dma_start(xdst(kq, 1, 0), xsrc(kq, 1, 0))
    for kq in range(4):
        nc.sync.dma_start(wdst(kq, 1), wsrc(kq, 1))
    for kq in range(4):
        nc.sync.dma_start(xdst(kq, 0, 1), xsrc(kq, 0, 1))
        nc.sync.dma_start(xdst(kq, 1, 1), xsrc(kq, 1, 1))
    for k in range(4):
        nc.sync.dma_start(wphl_sb[:, k, :, :],
                          wphl_d.ap()[k * P:(k + 1) * P, :].rearrange(
                              "p (two c) -> p two c", two=2))
    for n in range(2, NJ):
        for kq in range(4):
            nc.sync.dma_start(xdst(kq, 0, n), xsrc(kq, 0, n))
            nc.sync.dma_start(xdst(kq, 1, n), xsrc(kq, 1, n))

    # ---- constants ----
    make_identity(nc, ident[:])
    mkf = ynpool.tile([P, P], F32, tag="yn", name="mkf")
    nc.gpsimd.memset(mkf[:], 1.0)
    nc.gpsimd.affine_select(
        out=mkf[:], in_=mkf[:], compare_op=mybir.AluOpType.is_ge,
        fill=0.0, base=0, pattern=[[1, P]], channel_multiplier=-1)
    nc.scalar.copy(trimask[:], mkf[:])
    nc.vector.memset(v_sb[:], 1.0)  # ones columns; data cols overwritten below
    # pre-zero the score psum tiles: exp may read never-written columns in
    # diagonal windows (they are masked later), which must be finite
    for w in range(2):
        pwarm = pps.tile([P, 2 * TQ], F32, tag="ps", name="pswarm")
        nc.vector.memset(pwarm[:], 0.0)

    # ---- qkv projection units (fp8 h+l DoubleRow) ----
    def qkv_matmuls(ps_ap, wcol0, wcols, tcol0, tcols, swap=False):
        """Three DoubleRow chains -- (wh,xh), (wh,xl), (wl,xh) -- each
        contracting 2 k-tiles per instruction.  swap: x stationary."""
        for ci, (wp_, xp_) in enumerate([(0, 0), (0, 1), (1, 0)]):
            for kp in range(KC // 2):
                wap = whl_sb[:, 2 * kp:2 * kp + 2, wp_, wcol0:wcol0 + wcols]
                xap = xhl_sb[:, 2 * kp:2 * kp + 2, xp_, tcol0:tcol0 + tcols]
                lhsT, rhs = (xap, wap) if swap else (wap, xap)
                nc.tensor.matmul(ps_ap, lhsT, rhs,
                                 start=(ci == 0 and kp == 0),
                                 stop=(ci == 2 and kp == KC // 2 - 1),
                                 perf_mode=DR)

    def unit_q(n, mt):
        def go():
            ps = ppm.tile([P, TQ], F32, tag="pm", name="psq")
            qkv_matmuls(ps[:], mt * P, P, n * TQ, TQ)
            nc.vector.tensor_scalar(
                out=q_sb[:, mt * T + n * TQ: mt * T + (n + 1) * TQ],
                in0=ps[:], scalar1=1.0 / WSCALE, scalar2=bq_sb[:, mt:mt + 1],
                op0=mult, op1=add)
        return go

    def unit_k(n):
        def go():
            ps = ppm.tile([P, TQ], F32, tag="pm", name="psk")
            qkv_matmuls(ps[:], QROWS, P, n * TQ, TQ)
            nc.vector.tensor_scalar(
                out=kT_sb[:, n * TQ:(n + 1) * TQ],
                in0=ps[:], scalar1=0.125 / WSCALE, scalar2=bk_sb[:, 0:1],
                op0=mult, op1=add)
        return go

    def unit_v(i):
        # v_sb tile i: [0:64]=kv0, 64=ones, [65:129]=kv1, 129=ones
        def go():
            ps = ppm.tile([P, TQ], F32, tag="pm", name="psv")
            qkv_matmuls(ps[:, 0:P], QROWS + P, P, i * P, P, swap=True)
            nc.vector.tensor_scalar(
                out=v_sb[:, i * 130: i * 130 + 64], in0=ps[:, 0:64],
                scalar1=1.0 / WSCALE, scalar2=None, op0=mult)
            nc.vector.tensor_scalar(
                out=v_sb[:, i * 130 + 65: i * 130 + 129], in0=ps[:, 64:128],
                scalar1=1.0 / WSCALE, scalar2=None, op0=mult)
        return go

    # ---- attention units ----
    def unit_attn(j, hp):
        # head pair (hp, hp+4): q/y column tile `hp`, head A on partitions
        # 0:64 of q/kT (kv0), head B on 64:128 (kv1).
        # Main generator: one yield per i-tile "slot"; PV trails scores by
        # 2 i-tiles so each PV matmul's exp finished a full slot earlier
        # (the PE is in-order -- a waiting matmul blocks the queue).
        # The normalize/transpose tail is returned as a separate generator
        # that the scheduler interleaves with the NEXT unit's slots.
        nb = 4 * (j + 1)   # tk tiles in play (block-causal)
        qcol = hp * T + j * TQ
        pv = [None, None]
        pv_first = [True, True]
        pts = {}
        ys = [None, None]

        def emit_scores_exp(i):
            r = i - 4 * j
            c0 = max(0, r) * P
            ps = pps.tile([P, 2 * TQ], F32, tag="ps", name="pss")
            for h in (0, 1):
                rb = 64 * h
                nc.tensor.matmul(
                    ps[:, TQ * h + c0: TQ * (h + 1)],
                    kT_sb[rb:rb + 64, i * P:(i + 1) * P],
                    q_sb[rb:rb + 64, qcol + c0: qcol + TQ],
                    start=True, stop=True)
            pt = ptpool.tile([P, 2 * TQ], BF16, tag="pt", name="pt")
            pts[i] = pt
            if c0 >= 2 * P:
                # skip the head-B hole [TQ, TQ+c0) -- two exps are cheaper
                # than the wasted columns once c0 >= 256
                nc.scalar.activation(pt[:, c0:TQ], ps[:, c0:TQ], ExpF)
                nc.scalar.activation(pt[:, TQ + c0:2 * TQ], ps[:, TQ + c0:2 * TQ], ExpF)
            else:
                nc.scalar.activation(pt[:, c0:2 * TQ], ps[:, c0:2 * TQ], ExpF)
            if r >= 0:
                for h in (0, 1):
                    tri = pt[:, TQ * h + c0: TQ * h + c0 + P]
                    MASK_ENG.tensor_tensor(out=tri, in0=tri, in1=trimask[:], op=mult)

        def emit_pv(i):
            pt = pts.pop(i)
            r = i - 4 * j
            for h in (0, 1):
                for s in range(max(0, r), 4):
                    first = pv_first[h]
                    pv_first[h] = False
                    nc.tensor.matmul(
                        pv[h][:, 65 * s: 65 * s + 65],
                        pt[:, TQ * h + P * s: TQ * h + P * (s + 1)],
                        v_sb[:, i * 130 + 65 * h: i * 130 + 65 * h + 65],
                        start=first, stop=(i == nb - 1 and s == 3),
                        skip_group_check=not first)

        def main_gen():
            pv[0] = ppv.tile([P, 4 * 65], F32, tag="pv", name="pvA")
            pv[1] = ppv.tile([P, 4 * 65], F32, tag="pv", name="pvB")
            for i in range(nb):
                emit_scores_exp(i)
                if i >= PV_LAG:
                    emit_pv(i - PV_LAG)
                yield 650
            for i in range(max(0, nb - PV_LAG), nb - 1):
                emit_pv(i)
                yield 350
            emit_pv(nb - 1)
            # drain the PV psum banks so the next unit can claim them
            for h in (0, 1):
                ys[h] = yspool.tile([P, 4 * 65], F32, tag="ys", name=f"ys{h}")
                nc.vector.tensor_copy(ys[h][:], pv[h][:])
            yield 350

        def tail_gen():
            # normalize into bf16 (Pool), transpose in bf16 (PE), then h/l
            # fp8 split during the psum->sbuf copies (DVE).  fp8 values are
            # exact in bf16, so yh+yl reproduces the bf16 y exactly.
            for s in range(4):
                for h in (0, 1):
                    o = P * s + 64 * h + hp * TQ
                    nc.gpsimd.normalize_recip(
                        ynf[:, o: o + 64],
                        ys[h][:, 65 * s: 65 * s + 64],
                        ys[h][:, 65 * s + 64: 65 * s + 65])
                yield 0
            yield 0
            ptr = ppm.tile([P, TQ], BF16, tag="pm", name="ptr")
            for s in range(4):
                nc.tensor.matmul(
                    ptr[:, P * s: P * (s + 1)],
                    ynf[:, P * s + hp * TQ: P * (s + 1) + hp * TQ],
                    ident[:], is_transpose=True,
                    start=(s == 0), stop=(s == 3), skip_group_check=(s != 0))
            yh_dst = yhl_sb[:, 0, hp, j * TQ: (j + 1) * TQ]
            nc.vector.tensor_copy(yh_dst, ptr[:])
            nc.vector.tensor_tensor(
                out=yhl_sb[:, 1, hp, j * TQ: (j + 1) * TQ],
                in0=ptr[:], in1=yh_dst, op=mybir.AluOpType.subtract)
        return main_gen(), tail_gen

    def unit_cproj(j, ms, split_dma=False):
        def gen():
            os_t = outpool.tile([P, 4 * TQ], F32, tag="os", name="os")
            for n in range(NJ):
                pc = ppm.tile([P, TQ], F32, tag="pm", name="pc")
                for ci, (yp, wpp) in enumerate([(0, 0), (0, 1), (1, 0)]):
                    for kp in (0, 1):
                        nc.tensor.matmul(
                            pc[:],
                            yhl_sb[:, yp, 2 * kp:2 * kp + 2,
                                   j * TQ + ms * P: j * TQ + (ms + 1) * P],
                            wphl_sb[:, 2 * kp:2 * kp + 2, wpp,
                                    n * TQ:(n + 1) * TQ],
                            start=(ci == 0 and kp == 0),
                            stop=(ci == 2 and kp == 1), perf_mode=DR)
                    if ci == 1:
                        yield 330
                nc.vector.tensor_scalar(
                    out=os_t[:, n * TQ:(n + 1) * TQ], in0=pc[:],
                    scalar1=1.0 / WSCALE, scalar2=None, op0=mult)
                if split_dma:
                    nc.sync.dma_start(
                        out_d.ap()[j * TQ + ms * P: j * TQ + (ms + 1) * P,
                                   n * TQ:(n + 1) * TQ],
                        os_t[:, n * TQ:(n + 1) * TQ])
                yield 430
            if not split_dma:
                nc.sync.dma_start(
                    out_d.ap()[j * TQ + ms * P: j * TQ + (ms + 1) * P, :], os_t[:])
        return gen()

    def gen_q(n, mt):
        def gen():
            ps = ppm.tile([P, TQ], F32, tag="pm", name="psq")
            for seg in _qkv_segs(ps[:], mt * P, P, n * TQ, TQ):
                yield seg
            nc.vector.tensor_scalar(
                out=q_sb[:, mt * T + n * TQ: mt * T + (n + 1) * TQ],
                in0=ps[:], scalar1=1.0 / WSCALE, scalar2=bq_sb[:, mt:mt + 1],
                op0=mult, op1=add)
        return gen()

    def gen_k(n):
        def gen():
            ps = ppm.tile([P, TQ], F32, tag="pm", name="psk")
            for seg in _qkv_segs(ps[:], QROWS, P, n * TQ, TQ):
                yield seg
            nc.vector.tensor_scalar(
                out=kT_sb[:, n * TQ:(n + 1) * TQ],
                in0=ps[:], scalar1=0.125 / WSCALE, scalar2=bk_sb[:, 0:1],
                op0=mult, op1=add)
        return gen()

    def _qkv_segs(ps_ap, wcol0, wcols, tcol0, tcols, swap=False):
        emitted = 0
        for ci, (wp_, xp_) in enumerate([(0, 0), (0, 1), (1, 0)]):
            for kp in range(KC // 2):
                wap = whl_sb[:, 2 * kp:2 * kp + 2, wp_, wcol0:wcol0 + wcols]
                xap = xhl_sb[:, 2 * kp:2 * kp + 2, xp_, tcol0:tcol0 + tcols]
                lhsT, rhs = (xap, wap) if swap else (wap, xap)
                nc.tensor.matmul(ps_ap, lhsT, rhs,
                                 start=(ci == 0 and kp == 0),
                                 stop=(ci == 2 and kp == KC // 2 - 1),
                                 perf_mode=DR)
                emitted += 1
                if emitted % 6 == 0 and emitted < 24:
                    yield 640

    def gen_v(i):
        # v_sb tile i: [0:64]=kv0, 64=ones, [65:129]=kv1, 129=ones
        def gen():
            ps = ppm.tile([P, TQ], F32, tag="pm", name="psv")
            for seg in _qkv_segs(ps[:, 0:P], QROWS + P, P, i * P, P, swap=True):
                pass  # 27 ns per matmul; no need to split
            nc.vector.tensor_scalar(
                out=v_sb[:, i * 130: i * 130 + 64], in0=ps[:, 0:64],
                scalar1=1.0 / WSCALE, scalar2=None, op0=mult)
            nc.vector.tensor_scalar(
                out=v_sb[:, i * 130 + 65: i * 130 + 129], in0=ps[:, 64:128],
                scalar1=1.0 / WSCALE, scalar2=None, op0=mult)
            yield 650
        return gen()

    def proj_gens(n):
        return ([gen_q(n, mt) for mt in range(4)] + [gen_k(n)]
                + [gen_v(i) for i in range(4 * n, 4 * n + 4)])

    # yn staging buffers for normalize->transpose, one [P, TQ] region per pair
    ynf = persist.tile([P, 4 * TQ], BF16, tag="ynf")

    class FillerQueue:
        """Sequential queue of generator units; pulls ~budget ns of PE
        segments at a time."""

        def __init__(self):
            self.gens = []
            self.cur = None
            self.done = 0

        def add(self, gens):
            self.gens.extend(gens)

        def push_front(self, gen):
            self.gens.insert(0, gen)

        def pull(self, budget):
            got = 0
            while got < budget:
                if self.cur is None:
                    if not self.gens:
                        return got
                    self.cur = self.gens.pop(0)
                try:
                    got += next(self.cur)
                except StopIteration:
                    self.cur = None
                    self.done += 1
            return got

        def drain(self):
            while self.pull(1 << 30) > 0:
                pass

    # ---- software-pipelined emission ----
    # Three filler queues by priority: fqd (this window's deferred q m-tiles
    # 1-3 -- needed by attn units 1-3 of the SAME window), fqp (next window's
    # critical projections: q m-tile 0, k, v), fqf (cproj -- fully flexible,
    # carried across windows to feed the ACT-bound late windows).
    fqd = FillerQueue()
    fqp = FillerQueue()
    fqf = FillerQueue()

    def pull(budget):
        got = fqd.pull(budget)
        if got < budget:
            got += fqp.pull(budget - got)
        if got < budget:
            fqf.pull(budget - got)

    def step(g):
        if g is None:
            return None
        try:
            next(g)
            return g
        except StopIteration:
            return None

    # prologue: everything attn(0, hp=0) touches must be fully emitted
    # before its PV reads are emitted (emission order = dependency order)
    fqp.add([gen_q(0, 0), gen_k(0)] + [gen_v(i) for i in range(4)])
    fqp.drain()
    pending_tail = None
    deferred_q = {j: [gen_q(j, mt) for mt in (1, 2, 3)] for j in range(NJ)}
    for j in range(NJ):
        fqd.add(deferred_q[j])
        if j + 1 < NJ:
            fqp.add([gen_q(j + 1, 0), gen_k(j + 1)]
                    + [gen_v(i) for i in range(4 * (j + 1), 4 * (j + 1) + 4)])
        nslots = 4 * (4 * (j + 1) + 3)
        budget = BUDGETS[j] / nslots
        done0 = fqd.done
        for hp in range(4):
            if hp >= 1:
                # attn(j, hp) needs q(j, hp): force any unemitted remainder
                while (fqd.done - done0 < hp
                       and (fqd.cur is not None or fqd.gens)):
                    fqd.pull(700)
            main, tail = unit_attn(j, hp)
            for si, hint in enumerate(main):
                pull(budget)
                for _ in range(TAIL_STEPS):
                    pending_tail = step(pending_tail)
                if hp == 0 and si == 6:
                    # attn(j-1, 3)'s tail (stepped once per slot) has fully
                    # emitted its yhl write by slot 6; cproj may follow it
                    for (jj, ms) in CPROJ_AT.get(j, []):
                        fqf.add([unit_cproj(jj, ms)])
            pending_tail = tail()
        fqp.drain()
    while pending_tail is not None:
        pending_tail = step(pending_tail)
        pull(500)
    fqf.add([unit_cproj(3, ms, split_dma=True) for ms in range(4)])
    fqf.drain()


def _prep_inputs(x, w_attn, b_attn, w_proj):
    """Host-side shard + transpose + fp8 h+l split for each of the 8 cores."""
    in_maps = []
    xhl = {}
    for b in range(B):
        xT = np.ascontiguousarray(np.asarray(x[b], np.float32).T)
        xh = xT.astype(NPFP8)
        xl = (xT - xh.astype(np.float32)).astype(NPFP8)
        xhl[b] = np.concatenate([xh, xl], axis=1)  # [C, 2T]
    for g in range(N_CORES):
        b, grp = divmod(g, 4)

        q_rows = []
        for lh in Q_ORDER:
            gh = HL * grp + lh
            q_rows.extend(range(HS * gh, HS * gh + HS))
        k0 = NE + KROWS * grp
        v0 = NE + N_KV * HS + KROWS * grp
        rows = q_rows + list(range(k0, k0 + KROWS)) + list(range(v0, v0 + KROWS))
        wqkvT = np.ascontiguousarray(w_attn[rows, :].T) * WSCALE
        wh = wqkvT.astype(NPFP8)
        wl = (wqkvT - wh.astype(np.float32)).astype(NPFP8)
        whl = np.concatenate([wh, wl], axis=1)  # [C, 2*WCOLS]

        cols = []
        for lh in Q_ORDER:
            gh = HL * grp + lh
            cols.extend(range(HS * gh, HS * gh + HS))
        wpT = np.ascontiguousarray(w_proj[:, cols].T) * WSCALE
        wph = wpT.astype(NPFP8)
        wpl = (wpT - wph.astype(np.float32)).astype(NPFP8)
        wphl = np.concatenate([wph, wpl], axis=1)  # [QROWS, 2C]

        bq = np.asarray(b_attn[q_rows], np.float32).reshape(4, P)
        bk = (np.asarray(b_attn[k0:k0 + KROWS], np.float32) / 8.0).reshape(1, P)

        in_maps.append({"xhl": xhl[b], "whl": whl, "wphl": wphl,
                        "bq": bq, "bk": bk})
    return in_maps


def get_nc():
    if "nc" not in _CACHE:
        _CACHE["nc"] = _build_program()
    return _CACHE["nc"]


def kernel(x, w_attn, b_attn, w_proj, b_proj):
    x = np.asarray(x, np.float32)
    w_attn = np.asarray(w_attn, np.float32)
    b_attn = np.asarray(b_attn, np.float32)
    w_proj = np.asarray(w_proj, np.float32)
    b_proj = np.asarray(b_proj, np.float32)

    nc = get_nc()
    in_maps = _prep_inputs(x, w_attn, b_attn, w_proj)
    res = run_bass_kernel_spmd(nc, in_maps, core_ids=list(range(N_CORES)))

    # host "all-reduce" over the 4 head-group cores per batch + bias folds
    bv = b_attn[NE + N_KV * HS:]                      # [512] v bias
    bv_full = np.repeat(bv.reshape(N_KV, HS), N_HEAD // N_KV, axis=0).reshape(-1)
    delta = bv_full @ w_proj.T + b_proj               # [2048]
    out = np.zeros((B, T, C), np.float32)
    for g in range(N_CORES):
        b = g // 4
        out[b] += res.results[g]["out"]
    out += delta[None, None, :]
    return out


# revision 6
# speedup vs baseline: 1.0436x; 1.0131x over previous
"""Trainium2 Bass kernel for GQA causal self-attention (nn_CausalSelfAttention).

Model (hardcoded from the problem spec):
  B=2, T=2048, C=2048, n_head=32, n_kv=8, hs=64
  qkv = x @ w_attn.T + b_attn ; causal GQA attention ; y @ w_proj.T + b_proj

Sharding over 8 cores: core g handles batch b = g//4 and head-group grp = g%4
(8 q-heads, 2 kv-heads per core).  c_attn columns and c_proj rows are split
head-wise; the c_proj partial sums are reduced on the host (the "all-reduce").

v2 design notes (on top of the v1 baseline):
 - qkv projection in fp8e4m3 DoubleRow with an h+l (high + low residual)
   decomposition: x = xh + xl, 32*w = wh + wl, all fp8, split on the host.
   Three DoubleRow chains per output tile (wh@xh, wh@xl, wl@xh), each
   contracting 2 k-tiles per instruction = 0.75x of the bf16 PE cost at
   ~bf16 accuracy (xl@wl dropped).  The 1/32 weight prescale is undone in
   the psum->sbuf bias-add copy.
 - scores stay bf16, K-stationary (S.T tile [tk, tq]) as in v1.
 - causal masking multiplies only the 128-wide diagonal triangle blocks.
 - PV is "flipped": out y[tq, hs+1] with stationary pt-chunks [tk, 128],
   moving v [tk, 65] (64 dims + ones column -> softmax denominator).
   PE cost 65 per (i-tile, head, tq-subtile) vs 512 in the [hs, tq]
   orientation.  The 8 per-head [128, 65] accumulators of a head share
   one PSUM bank via the per-byte pending-zero protocol: the first matmul
   of a bank starts the group, later slices use start=False +
   skip_group_check (their first write lands on pending-zero bytes and
   overwrites; subsequent writes accumulate).
 - softmax normalization: gpsimd normalize_recip (Pool engine) divides
   y[tq, hs] rows by the denominator column, writing bf16 directly.
 - y is transposed back to [hs, tq] with PE transposes (4 transposes of
   one head pair packed into one psum bank) for the c_proj matmul.
"""

import sys
import numpy as np
import ml_dtypes
from contextlib import ExitStack

for _p in ("/opt/trn_rl_repo", "/root/.axon_site/_ro/trn_rl_repo"):
    if _p not in sys.path:
        sys.path.append(_p)

import concourse.mybir as mybir
import concourse.tile as tile
from concourse import bacc
from concourse.bass_utils import run_bass_kernel_spmd
from concourse.masks import make_identity

BF16 = mybir.dt.bfloat16
F32 = mybir.dt.float32
FP8 = mybir.dt.float8e4
NPBF16 = ml_dtypes.bfloat16
NPFP8 = ml_dtypes.float8_e4m3
DR = mybir.MatmulPerfMode.DoubleRow

B, T, C = 2, 2048, 2048
N_HEAD, N_KV, HS = 32, 8, 64
NE = 2048
N_CORES = 8
HL = 8          # q heads per core
KVL = 2         # kv heads per core
P = 128
TQ = 512        # tq window (matmul moving width)
NJ = T // TQ    # 4 tq windows
NT = T // P     # 16 token tiles
KC = C // P     # 16 contraction tiles over channels
QROWS = HL * HS          # 512 local q rows
KROWS = KVL * HS         # 128 local k rows
WCOLS = QROWS + 2 * KROWS  # 768 local w_attn rows
WSCALE = 32.0   # host prescale on w_attn so fp8 residuals stay in range

# position-block -> local head: q_sb m-tile mt rows [0:64]=head mt, [64:128]=head mt+4
Q_ORDER = [0, 4, 1, 5, 2, 6, 3, 7]

_CACHE = {}

# scheduling knobs (overridable before get_nc())
BUDGETS = [23200, 15400, 25600, 20500]
TAIL_STEPS = 1
TAIL_FIRST = False
PV_LAG = 5
PT_BUFS = 8
MASK_ON_POOL = False
YS_BUFS = 8
OS_BUFS = 3
INLINE_TRAIL_W = {0, 1, 2, 3}
CPROJ_AT = {2: [(0, ms) for ms in range(4)],
            3: [(jj, ms) for jj in (1, 2) for ms in range(4)]}


def _build_program():
    nc = bacc.Bacc("TRN2", target_bir_lowering=False, debug=False)

    xhl_d = nc.dram_tensor("xhl", [C, 2 * T], FP8, kind="ExternalInput")
    whl_d = nc.dram_tensor("whl", [C, 2 * WCOLS], FP8, kind="ExternalInput")
    wphl_d = nc.dram_tensor("wphl", [QROWS, 2 * C], FP8, kind="ExternalInput")
    bq_d = nc.dram_tensor("bq", [4, P], F32, kind="ExternalInput")
    bk_d = nc.dram_tensor("bk", [1, P], F32, kind="ExternalInput")
    out_d = nc.dram_tensor("out", [T, C], F32, kind="ExternalOutput")

    with tile.TileContext(nc) as tc:
        with ExitStack() as ctx:
            _emit(ctx, tc, nc, xhl_d, whl_d, wphl_d, bq_d, bk_d, out_d)
    nc.compile()
    return nc


def _emit(ctx, tc, nc, xhl_d, whl_d, wphl_d, bq_d, bk_d, out_d):
    MASK_ENG = nc.gpsimd if MASK_ON_POOL else nc.vector
    ExpF = mybir.ActivationFunctionType.Exp
    add = mybir.AluOpType.add
    mult = mybir.AluOpType.mult

    persist = ctx.enter_context(tc.tile_pool(name="persist", bufs=1))
    pps = ctx.enter_context(tc.tile_pool(name="pps", bufs=2, space="PSUM"))
    ppv = ctx.enter_context(tc.tile_pool(name="ppv", bufs=2, space="PSUM"))
    ppm = ctx.enter_context(tc.tile_pool(name="ppm", bufs=2, space="PSUM"))
    ptpool = ctx.enter_context(tc.tile_pool(name="pt", bufs=PT_BUFS))
    yspool = ctx.enter_context(tc.tile_pool(name="ys", bufs=YS_BUFS))
    ynpool = ctx.enter_context(tc.tile_pool(name="yn", bufs=4))
    outpool = ctx.enter_context(tc.tile_pool(name="os", bufs=OS_BUFS))

    # ---- persistent SBUF tensors ----
    # plane dim: 0 = h (fp8 high), 1 = l (fp8 residual)
    xhl_sb = persist.tile([P, KC, 2, T], FP8, tag="xhl")
    whl_sb = persist.tile([P, KC, 2, WCOLS], FP8, tag="whl")
    wphl_sb = persist.tile([P, 4, 2, C], FP8, tag="wphl")
    q_sb = persist.tile([P, 4 * T], BF16, tag="q")
    kT_sb = persist.tile([P, T], BF16, tag="k")
    v_sb = persist.tile([P, NT * 130], BF16, tag="v")
    yhl_sb = persist.tile([P, 2, 4, T], FP8, tag="yhl")
    bq_sb = persist.tile([P, 4], F32, tag="bq")
    bk_sb = persist.tile([P, 1], F32, tag="bk")
    ident = persist.tile([P, P], BF16, tag="ident")
    # triangle mask for diagonal blocks: trimask[x, y] = 1 if y >= x else 0
    trimask = persist.tile([P, P], BF16, tag="trimask")

    # ---- input DMAs ----
    # One DMA per (k-tile [, token chunk]): every DMA serializes ~630 ns on
    # the shared HWDGE device, so fewer/bigger transfers pace the startup.
    # Emission order matches consumption: w, x chunk 0, chunk 1, wp (needed
    # by cproj(0) during window 1), chunks 2-3.
    def wsrc(kq, pl):
        # 4 k-tiles of one w plane: [128, 4, WCOLS]
        return whl_d.ap()[4 * kq * P:(4 * kq + 4) * P, :].rearrange(
            "(kk p) (two w) -> p kk two w", p=P, two=2)[:, :, pl, :]

    def xsrc(kq, pl, n):
        return xhl_d.ap()[4 * kq * P:(4 * kq + 4) * P, :].rearrange(
            "(kk p) (two t) -> p kk two t", p=P, two=2)[:, :, pl, n * TQ:(n + 1) * TQ]

    def wdst(kq, pl):
        return whl_sb[:, 4 * kq:4 * kq + 4, pl, :]

    def xdst(kq, pl, n):
        return xhl_sb[:, 4 * kq:4 * kq + 4, pl, n * TQ:(n + 1) * TQ]

    # order matches chain consumption: (wh,xh) mains, then xl, then wl
    for kq in range(4):
        nc.sync.dma_start(wdst(kq, 0), wsrc(kq, 0))
        nc.sync.dma_start(xdst(kq, 0, 0), xsrc(kq, 0, 0))
    nc.sync.dma_start(bq_sb[:], bq_d.ap().rearrange("t p -> p t"))
    nc.sync.dma_start(bk_sb[:], bk_d.ap().rearrange("t p -> p t"))
    for kq in range(4):
        nc.sync.dma_start(xdst(kq, 1, 0), xsrc(kq, 1, 0))
    for kq in range(4):
        nc.sync.dma_start(wdst(kq, 1), wsrc(kq, 1))
    for kq in range(4):
        nc.sync.dma_start(xdst(kq, 0, 1), xsrc(kq, 0, 1))
        nc.sync.dma_start(xdst(kq, 1, 1), xsrc(kq, 1, 1))
    for k in range(4):
        nc.sync.dma_start(wphl_sb[:, k, :, :],
                          wphl_d.ap()[k * P:(k + 1) * P, :].rearrange(
                              "p (two c) -> p two c", two=2))
    for n in range(2, NJ):
        for kq in range(4):
            nc.sync.dma_start(xdst(kq, 0, n), xsrc(kq, 0, n))
            nc.sync.dma_start(xdst(kq, 1, n), xsrc(kq, 1, n))

    # ---- constants ----
    make_identity(nc, ident[:])
    mkf = ynpool.tile([P, P], F32, tag="yn", name="mkf")
    nc.gpsimd.memset(mkf[:], 1.0)
    nc.gpsimd.affine_select(
        out=mkf[:], in_=mkf[:], compare_op=mybir.AluOpType.is_ge,
        fill=0.0, base=0, pattern=[[1, P]], channel_multiplier=-1)
    nc.scalar.copy(trimask[:], mkf[:])
    nc.vector.memset(v_sb[:], 1.0)  # ones columns; data cols overwritten below
    # pre-zero the score psum tiles: exp may read never-written columns in
    # diagonal windows (they are masked later), which must be finite
    for w in range(2):
        pwarm = pps.tile([P, 2 * TQ], F32, tag="ps", name="pswarm")
        nc.vector.memset(pwarm[:], 0.0)

    # ---- qkv projection units (fp8 h+l DoubleRow) ----
    def qkv_matmuls(ps_ap, wcol0, wcols, tcol0, tcols, swap=False):
        """Three DoubleRow chains -- (wh,xh), (wh,xl), (wl,xh) -- each
        contracting 2 k-tiles per instruction.  swap: x stationary."""
        for ci, (wp_, xp_) in enumerate([(0, 0), (0, 1), (1, 0)]):
            for kp in range(KC // 2):
                wap = whl_sb[:, 2 * kp:2 * kp + 2, wp_, wcol0:wcol0 + wcols]
                xap = xhl_sb[:, 2 * kp:2 * kp + 2, xp_, tcol0:tcol0 + tcols]
                lhsT, rhs = (xap, wap) if swap else (wap, xap)
                nc.tensor.matmul(ps_ap, lhsT, rhs,
                                 start=(ci == 0 and kp == 0),
                                 stop=(ci == 2 and kp == KC // 2 - 1),
                                 perf_mode=DR)

    def unit_q(n, mt):
        def go():
            ps = ppm.tile([P, TQ], F32, tag="pm", name="psq")
            qkv_matmuls(ps[:], mt * P, P, n * TQ, TQ)
            nc.vector.tensor_scalar(
                out=q_sb[:, mt * T + n * TQ: mt * T + (n + 1) * TQ],
                in0=ps[:], scalar1=1.0 / WSCALE, scalar2=bq_sb[:, mt:mt + 1],
                op0=mult, op1=add)
        return go

    def unit_k(n):
        def go():
            ps = ppm.tile([P, TQ], F32, tag="pm", name="psk")
            qkv_matmuls(ps[:], QROWS, P, n * TQ, TQ)
            nc.vector.tensor_scalar(
                out=kT_sb[:, n * TQ:(n + 1) * TQ],
                in0=ps[:], scalar1=0.125 / WSCALE, scalar2=bk_sb[:, 0:1],
                op0=mult, op1=add)
        return go

    def unit_v(i):
        # v_sb tile i: [0:64]=kv0, 64=ones, [65:129]=kv1, 129=ones
        def go():
            ps = ppm.tile([P, TQ], F32, tag="pm", name="psv")
            qkv_matmuls(ps[:, 0:P], QROWS + P, P, i * P, P, swap=True)
            nc.vector.tensor_scalar(
                out=v_sb[:, i * 130: i * 130 + 64], in0=ps[:, 0:64],
                scalar1=1.0 / WSCALE, scalar2=None, op0=mult)
            nc.vector.tensor_scalar(
                out=v_sb[:, i * 130 + 65: i * 130 + 129], in0=ps[:, 64:128],
                scalar1=1.0 / WSCALE, scalar2=None, op0=mult)
        return go

    # ---- attention units ----
    def unit_attn(j, hp):
        # head pair (hp, hp+4): q/y column tile `hp`, head A on partitions
        # 0:64 of q/kT (kv0), head B on 64:128 (kv1).
        # Main generator: one yield per i-tile "slot"; PV trails scores by
        # 2 i-tiles so each PV matmul's exp finished a full slot earlier
        # (the PE is in-order -- a waiting matmul blocks the queue).
        # The normalize/transpose tail is returned as a separate generator
        # that the scheduler interleaves with the NEXT unit's slots.
        nb = 4 * (j + 1)   # tk tiles in play (block-causal)
        qcol = hp * T + j * TQ
        pv = [None, None]
        pv_first = [True, True]
        pts = {}
        ys = [None, None]

        def emit_scores_exp(i):
            r = i - 4 * j
            c0 = max(0, r) * P
            ps = pps.tile([P, 2 * TQ], F32, tag="ps", name="pss")
            for h in (0, 1):
                rb = 64 * h
                nc.tensor.matmul(
                    ps[:, TQ * h + c0: TQ * (h + 1)],
                    kT_sb[rb:rb + 64, i * P:(i + 1) * P],
                    q_sb[rb:rb + 64, qcol + c0: qcol + TQ],
                    start=True, stop=True)
            pt = ptpool.tile([P, 2 * TQ], BF16, tag="pt", name="pt")
            pts[i] = pt
            if c0 >= 2 * P:
                # skip the head-B hole [TQ, TQ+c0) -- two exps are cheaper
                # than the wasted columns once c0 >= 256
                nc.scalar.activation(pt[:, c0:TQ], ps[:, c0:TQ], ExpF)
                nc.scalar.activation(pt[:, TQ + c0:2 * TQ], ps[:, TQ + c0:2 * TQ], ExpF)
            else:
                nc.scalar.activation(pt[:, c0:2 * TQ], ps[:, c0:2 * TQ], ExpF)
            if r >= 0:
                for h in (0, 1):
                    tri = pt[:, TQ * h + c0: TQ * h + c0 + P]
                    MASK_ENG.tensor_tensor(out=tri, in0=tri, in1=trimask[:], op=mult)

        def emit_pv(i):
            pt = pts.pop(i)
            r = i - 4 * j
            for h in (0, 1):
                for s in range(max(0, r), 4):
                    first = pv_first[h]
                    pv_first[h] = False
                    nc.tensor.matmul(
                        pv[h][:, 65 * s: 65 * s + 65],
                        pt[:, TQ * h + P * s: TQ * h + P * (s + 1)],
                        v_sb[:, i * 130 + 65 * h: i * 130 + 65 * h + 65],
                        start=first, stop=(i == nb - 1 and s == 3),
                        skip_group_check=not first)

        def main_gen():
            pv[0] = ppv.tile([P, 4 * 65], F32, tag="pv", name="pvA")
            pv[1] = ppv.tile([P, 4 * 65], F32, tag="pv", name="pvB")
            for i in range(nb):
                emit_scores_exp(i)
                if i >= PV_LAG:
                    emit_pv(i - PV_LAG)
                yield 650
            if j in INLINE_TRAIL_W:
                # emit trailing PVs + drains inline: the next unit's
                # scores/exp reach the (ACT-bound) pipeline sooner
                for i in range(max(0, nb - PV_LAG), nb):
                    emit_pv(i)
                for h in (0, 1):
                    ys[h] = yspool.tile([P, 4 * 65], F32, tag="ys", name=f"ys{h}")
                    nc.vector.tensor_copy(ys[h][:], pv[h][:])
                yield 650
            else:
                for i in range(max(0, nb - PV_LAG), nb - 1):
                    emit_pv(i)
                    yield 350
                emit_pv(nb - 1)
                # drain the PV psum banks so the next unit can claim them
                for h in (0, 1):
                    ys[h] = yspool.tile([P, 4 * 65], F32, tag="ys", name=f"ys{h}")
                    nc.vector.tensor_copy(ys[h][:], pv[h][:])
                yield 350

        def tail_gen():
            # normalize into bf16 (Pool), transpose in bf16 (PE), then h/l
            # fp8 split during the psum->sbuf copies (DVE).  fp8 values are
            # exact in bf16, so yh+yl reproduces the bf16 y exactly.
            for s in range(4):
                for h in (0, 1):
                    o = P * s + 64 * h + hp * TQ
                    nc.gpsimd.normalize_recip(
                        ynf[:, o: o + 64],
                        ys[h][:, 65 * s: 65 * s + 64],
                        ys[h][:, 65 * s + 64: 65 * s + 65])
                yield 0
            yield 0
            ptr = ppm.tile([P, TQ], BF16, tag="pm", name="ptr")
            for s in range(4):
                nc.tensor.matmul(
                    ptr[:, P * s: P * (s + 1)],
                    ynf[:, P * s + hp * TQ: P * (s + 1) + hp * TQ],
                    ident[:], is_transpose=True,
                    start=(s == 0), stop=(s == 3), skip_group_check=(s != 0))
            yh_dst = yhl_sb[:, 0, hp, j * TQ: (j + 1) * TQ]
            nc.vector.tensor_copy(yh_dst, ptr[:])
            nc.vector.tensor_tensor(
                out=yhl_sb[:, 1, hp, j * TQ: (j + 1) * TQ],
                in0=ptr[:], in1=yh_dst, op=mybir.AluOpType.subtract)
        return main_gen(), tail_gen

    def unit_cproj(j, ms, split_dma=False):
        def gen():
            os_t = outpool.tile([P, 4 * TQ], F32, tag="os", name="os")
            for n in range(NJ):
                pc = ppm.tile([P, TQ], F32, tag="pm", name="pc")
                for ii, (kp, (yp, wpp)) in enumerate(
                        [(kp, c) for kp in (0, 1)
                         for c in ((0, 0), (0, 1), (1, 0))]):
                    nc.tensor.matmul(
                        pc[:],
                        yhl_sb[:, yp, 2 * kp:2 * kp + 2,
                               j * TQ + ms * P: j * TQ + (ms + 1) * P],
                        wphl_sb[:, 2 * kp:2 * kp + 2, wpp,
                                n * TQ:(n + 1) * TQ],
                        start=(ii == 0), stop=(ii == 5), perf_mode=DR)
                    if ii == 2:
                        yield 330
                nc.vector.tensor_scalar(
                    out=os_t[:, n * TQ:(n + 1) * TQ], in0=pc[:],
                    scalar1=1.0 / WSCALE, scalar2=None, op0=mult)
                if split_dma:
                    nc.sync.dma_start(
                        out_d.ap()[j * TQ + ms * P: j * TQ + (ms + 1) * P,
                                   n * TQ:(n + 1) * TQ],
                        os_t[:, n * TQ:(n + 1) * TQ])
                yield 430
            if not split_dma:
                nc.sync.dma_start(
                    out_d.ap()[j * TQ + ms * P: j * TQ + (ms + 1) * P, :], os_t[:])
        return gen()

    def gen_q(n, mt):
        def gen():
            ps = ppm.tile([P, TQ], F32, tag="pm", name="psq")
            for seg in _qkv_segs(ps[:], mt * P, P, n * TQ, TQ):
                yield seg
            nc.vector.tensor_scalar(
                out=q_sb[:, mt * T + n * TQ: mt * T + (n + 1) * TQ],
                in0=ps[:], scalar1=1.0 / WSCALE, scalar2=bq_sb[:, mt:mt + 1],
                op0=mult, op1=add)
        return gen()

    def gen_k(n):
        def gen():
            ps = ppm.tile([P, TQ], F32, tag="pm", name="psk")
            for seg in _qkv_segs(ps[:], QROWS, P, n * TQ, TQ):
                yield seg
            nc.vector.tensor_scalar(
                out=kT_sb[:, n * TQ:(n + 1) * TQ],
                in0=ps[:], scalar1=0.125 / WSCALE, scalar2=bk_sb[:, 0:1],
                op0=mult, op1=add)
        return gen()

    def _qkv_segs(ps_ap, wcol0, wcols, tcol0, tcols, swap=False):
        emitted = 0
        for ci, (wp_, xp_) in enumerate([(0, 0), (0, 1), (1, 0)]):
            for kp in range(KC // 2):
                wap = whl_sb[:, 2 * kp:2 * kp + 2, wp_, wcol0:wcol0 + wcols]
                xap = xhl_sb[:, 2 * kp:2 * kp + 2, xp_, tcol0:tcol0 + tcols]
                lhsT, rhs = (xap, wap) if swap else (wap, xap)
                nc.tensor.matmul(ps_ap, lhsT, rhs,
                                 start=(ci == 0 and kp == 0),
                                 stop=(ci == 2 and kp == KC // 2 - 1),
                                 perf_mode=DR)
                emitted += 1
                if emitted % 6 == 0 and emitted < 24:
                    yield 640

    def gen_v(i):
        # v_sb tile i: [0:64]=kv0, 64=ones, [65:129]=kv1, 129=ones
        def gen():
            ps = ppm.tile([P, TQ], F32, tag="pm", name="psv")
            for seg in _qkv_segs(ps[:, 0:P], QROWS + P, P, i * P, P, swap=True):
                pass  # 27 ns per matmul; no need to split
            nc.vector.tensor_scalar(
                out=v_sb[:, i * 130: i * 130 + 64], in0=ps[:, 0:64],
                scalar1=1.0 / WSCALE, scalar2=None, op0=mult)
            nc.vector.tensor_scalar(
                out=v_sb[:, i * 130 + 65: i * 130 + 129], in0=ps[:, 64:128],
                scalar1=1.0 / WSCALE, scalar2=None, op0=mult)
            yield 650
        return gen()

    def proj_gens(n):
        return ([gen_q(n, mt) for mt in range(4)] + [gen_k(n)]
                + [gen_v(i) for i in range(4 * n, 4 * n + 4)])

    # yn staging buffers for normalize->transpose, one [P, TQ] region per pair
    ynf = persist.tile([P, 4 * TQ], BF16, tag="ynf")

    class FillerQueue:
        """Sequential queue of generator units; pulls ~budget ns of PE
        segments at a time."""

        def __init__(self):
            self.gens = []
            self.cur = None
            self.done = 0

        def add(self, gens):
            self.gens.extend(gens)

        def push_front(self, gen):
            self.gens.insert(0, gen)

        def pull(self, budget):
            got = 0
            while got < budget:
                if self.cur is None:
                    if not self.gens:
                        return got
                    self.cur = self.gens.pop(0)
                try:
                    got += next(self.cur)
                except StopIteration:
                    self.cur = None
                    self.done += 1
            return got

        def drain(self):
            while self.pull(1 << 30) > 0:
                pass

    # ---- software-pipelined emission ----
    # Three filler queues by priority: fqd (this window's deferred q m-tiles
    # 1-3 -- needed by attn units 1-3 of the SAME window), fqp (next window's
    # critical projections: q m-tile 0, k, v), fqf (cproj -- fully flexible,
    # carried across windows to feed the ACT-bound late windows).
    fqd = FillerQueue()
    fqp = FillerQueue()
    fqf = FillerQueue()

    def pull(budget):
        got = fqd.pull(budget)
        if got < budget:
            got += fqp.pull(budget - got)
        if got < budget:
            fqf.pull(budget - got)

    def step(g):
        if g is None:
            return None
        try:
            next(g)
            return g
        except StopIteration:
            return None

    # prologue: everything attn(0, hp=0) touches must be fully emitted
    # before its PV reads are emitted (emission order = dependency order)
    fqp.add([gen_q(0, 0), gen_k(0)] + [gen_v(i) for i in range(4)])
    fqp.drain()
    pending_tail = None
    deferred_q = {j: [gen_q(j, mt) for mt in (1, 2, 3)] for j in range(NJ)}
    for j in range(NJ):
        fqd.add(deferred_q[j])
        if j + 1 < NJ:
            fqp.add([gen_q(j + 1, 0), gen_k(j + 1)]
                    + [gen_v(i) for i in range(4 * (j + 1), 4 * (j + 1) + 4)])
        nslots = 4 * (4 * (j + 1) + 3)
        budget = BUDGETS[j] / nslots
        done0 = fqd.done
        for hp in range(4):
            if hp >= 1:
                # attn(j, hp) needs q(j, hp): force any unemitted remainder
                while (fqd.done - done0 < hp
                       and (fqd.cur is not None or fqd.gens)):
                    fqd.pull(700)
            main, tail = unit_attn(j, hp)
            for si, hint in enumerate(main):
                pull(budget)
                for _ in range(TAIL_STEPS):
                    pending_tail = step(pending_tail)
                if hp == 0 and si == 4:
                    # attn(j-1, 3)'s tail has fully emitted its yhl write
                    # (guaranteed by the flush below); cproj may follow it
                    for (jj, ms) in CPROJ_AT.get(j, []):
                        fqf.add([unit_cproj(jj, ms)])
            # never drop an unfinished tail: its yhl copy must be emitted
            while pending_tail is not None:
                pending_tail = step(pending_tail)
            pending_tail = tail()
        fqp.drain()
    while pending_tail is not None:
        pending_tail = step(pending_tail)
        pull(500)
    fqf.add([unit_cproj(3, ms, split_dma=True) for ms in range(4)])
    fqf.drain()


def _prep_inputs(x, w_attn, b_attn, w_proj):
    """Host-side shard + transpose + fp8 h+l split for each of the 8 cores."""
    in_maps = []
    xhl = {}
    for b in range(B):
        xT = np.ascontiguousarray(np.asarray(x[b], np.float32).T)
        xh = xT.astype(NPFP8)
        xl = (xT - xh.astype(np.float32)).astype(NPFP8)
        xhl[b] = np.concatenate([xh, xl], axis=1)  # [C, 2T]
    for g in range(N_CORES):
        b, grp = divmod(g, 4)

        q_rows = []
        for lh in Q_ORDER:
            gh = HL * grp + lh
            q_rows.extend(range(HS * gh, HS * gh + HS))
        k0 = NE + KROWS * grp
        v0 = NE + N_KV * HS + KROWS * grp
        rows = q_rows + list(range(k0, k0 + KROWS)) + list(range(v0, v0 + KROWS))
        wqkvT = np.ascontiguousarray(w_attn[rows, :].T) * WSCALE
        wh = wqkvT.astype(NPFP8)
        wl = (wqkvT - wh.astype(np.float32)).astype(NPFP8)
        whl = np.concatenate([wh, wl], axis=1)  # [C, 2*WCOLS]

        cols = []
        for lh in Q_ORDER:
            gh = HL * grp + lh
            cols.extend(range(HS * gh, HS * gh + HS))
        wpT = np.ascontiguousarray(w_proj[:, cols].T) * WSCALE
        wph = wpT.astype(NPFP8)
        wpl = (wpT - wph.astype(np.float32)).astype(NPFP8)
        wphl = np.concatenate([wph, wpl], axis=1)  # [QROWS, 2C]

        bq = np.asarray(b_attn[q_rows], np.float32).reshape(4, P)
        bk = (np.asarray(b_attn[k0:k0 + KROWS], np.float32) / 8.0).reshape(1, P)

        in_maps.append({"xhl": xhl[b], "whl": whl, "wphl": wphl,
                        "bq": bq, "bk": bk})
    return in_maps


def get_nc():
    if "nc" not in _CACHE:
        _CACHE["nc"] = _build_program()
    return _CACHE["nc"]


def kernel(x, w_attn, b_attn, w_proj, b_proj):
    x = np.asarray(x, np.float32)
    w_attn = np.asarray(w_attn, np.float32)
    b_attn = np.asarray(b_attn, np.float32)
    w_proj = np.asarray(w_proj, np.float32)
    b_proj = np.asarray(b_proj, np.float32)

    nc = get_nc()
    in_maps = _prep_inputs(x, w_attn, b_attn, w_proj)
    res = run_bass_kernel_spmd(nc, in_maps, core_ids=list(range(N_CORES)))

    # host "all-reduce" over the 4 head-group cores per batch + bias folds
    bv = b_attn[NE + N_KV * HS:]                      # [512] v bias
    bv_full = np.repeat(bv.reshape(N_KV, HS), N_HEAD // N_KV, axis=0).reshape(-1)
    delta = bv_full @ w_proj.T + b_proj               # [2048]
    out = np.zeros((B, T, C), np.float32)
    for g in range(N_CORES):
        b = g // 4
        out[b] += res.results[g]["out"]
    out += delta[None, None, :]
    return out


# revision 7
# speedup vs baseline: 1.0453x; 1.0017x over previous
"""Trainium2 Bass kernel for GQA causal self-attention (nn_CausalSelfAttention).

Model (hardcoded from the problem spec):
  B=2, T=2048, C=2048, n_head=32, n_kv=8, hs=64
  qkv = x @ w_attn.T + b_attn ; causal GQA attention ; y @ w_proj.T + b_proj

Sharding over 8 cores: core g handles batch b = g//4 and head-group grp = g%4
(8 q-heads, 2 kv-heads per core).  c_attn columns and c_proj rows are split
head-wise; the c_proj partial sums are reduced on the host (the "all-reduce").

v2 design notes (on top of the v1 baseline):
 - qkv projection in fp8e4m3 DoubleRow with an h+l (high + low residual)
   decomposition: x = xh + xl, 32*w = wh + wl, all fp8, split on the host.
   Three DoubleRow chains per output tile (wh@xh, wh@xl, wl@xh), each
   contracting 2 k-tiles per instruction = 0.75x of the bf16 PE cost at
   ~bf16 accuracy (xl@wl dropped).  The 1/32 weight prescale is undone in
   the psum->sbuf bias-add copy.
 - scores stay bf16, K-stationary (S.T tile [tk, tq]) as in v1.
 - causal masking multiplies only the 128-wide diagonal triangle blocks.
 - PV is "flipped": out y[tq, hs+1] with stationary pt-chunks [tk, 128],
   moving v [tk, 65] (64 dims + ones column -> softmax denominator).
   PE cost 65 per (i-tile, head, tq-subtile) vs 512 in the [hs, tq]
   orientation.  The 8 per-head [128, 65] accumulators of a head share
   one PSUM bank via the per-byte pending-zero protocol: the first matmul
   of a bank starts the group, later slices use start=False +
   skip_group_check (their first write lands on pending-zero bytes and
   overwrites; subsequent writes accumulate).
 - softmax normalization: gpsimd normalize_recip (Pool engine) divides
   y[tq, hs] rows by the denominator column, writing bf16 directly.
 - y is transposed back to [hs, tq] with PE transposes (4 transposes of
   one head pair packed into one psum bank) for the c_proj matmul.
"""

import sys
import numpy as np
import ml_dtypes
from contextlib import ExitStack

for _p in ("/opt/trn_rl_repo", "/root/.axon_site/_ro/trn_rl_repo"):
    if _p not in sys.path:
        sys.path.append(_p)

import concourse.mybir as mybir
import concourse.tile as tile
from concourse import bacc
from concourse.bass_utils import run_bass_kernel_spmd
from concourse.masks import make_identity

BF16 = mybir.dt.bfloat16
F32 = mybir.dt.float32
FP8 = mybir.dt.float8e4
NPBF16 = ml_dtypes.bfloat16
NPFP8 = ml_dtypes.float8_e4m3
DR = mybir.MatmulPerfMode.DoubleRow

B, T, C = 2, 2048, 2048
N_HEAD, N_KV, HS = 32, 8, 64
NE = 2048
N_CORES = 8
HL = 8          # q heads per core
KVL = 2         # kv heads per core
P = 128
TQ = 512        # tq window (matmul moving width)
NJ = T // TQ    # 4 tq windows
NT = T // P     # 16 token tiles
KC = C // P     # 16 contraction tiles over channels
QROWS = HL * HS          # 512 local q rows
KROWS = KVL * HS         # 128 local k rows
WCOLS = QROWS + 2 * KROWS  # 768 local w_attn rows
WSCALE = 32.0   # host prescale on w_attn so fp8 residuals stay in range

# position-block -> local head: q_sb m-tile mt rows [0:64]=head mt, [64:128]=head mt+4
Q_ORDER = [0, 4, 1, 5, 2, 6, 3, 7]

_CACHE = {}

# scheduling knobs (overridable before get_nc())
BUDGETS = [23200, 15400, 25600, 20500]
TAIL_STEPS = 1
TAIL_FIRST = False
PV_LAG = 5
PT_BUFS = 8
MASK_ON_POOL = False
YS_BUFS = 8
OS_BUFS = 3
INLINE_TRAIL_W = {0, 1, 2, 3}
CPROJ_AT = {2: [(0, ms) for ms in range(4)],
            3: [(jj, ms) for jj in (1, 2) for ms in range(4)]}


def _build_program():
    nc = bacc.Bacc("TRN2", target_bir_lowering=False, debug=False)

    xhl_d = nc.dram_tensor("xhl", [C, 2 * T], FP8, kind="ExternalInput")
    whl_d = nc.dram_tensor("whl", [C, 2 * WCOLS], FP8, kind="ExternalInput")
    wphl_d = nc.dram_tensor("wphl", [QROWS, 2 * C], FP8, kind="ExternalInput")
    bq_d = nc.dram_tensor("bq", [4, P], F32, kind="ExternalInput")
    bk_d = nc.dram_tensor("bk", [1, P], F32, kind="ExternalInput")
    out_d = nc.dram_tensor("out", [T, C], F32, kind="ExternalOutput")

    with tile.TileContext(nc) as tc:
        with ExitStack() as ctx:
            _emit(ctx, tc, nc, xhl_d, whl_d, wphl_d, bq_d, bk_d, out_d)
    nc.compile()
    return nc


def _emit(ctx, tc, nc, xhl_d, whl_d, wphl_d, bq_d, bk_d, out_d):
    MASK_ENG = nc.gpsimd if MASK_ON_POOL else nc.vector
    ExpF = mybir.ActivationFunctionType.Exp
    add = mybir.AluOpType.add
    mult = mybir.AluOpType.mult

    persist = ctx.enter_context(tc.tile_pool(name="persist", bufs=1))
    pps = ctx.enter_context(tc.tile_pool(name="pps", bufs=2, space="PSUM"))
    ppv = ctx.enter_context(tc.tile_pool(name="ppv", bufs=2, space="PSUM"))
    ppm = ctx.enter_context(tc.tile_pool(name="ppm", bufs=2, space="PSUM"))
    ptpool = ctx.enter_context(tc.tile_pool(name="pt", bufs=PT_BUFS))
    yspool = ctx.enter_context(tc.tile_pool(name="ys", bufs=YS_BUFS))
    ynpool = ctx.enter_context(tc.tile_pool(name="yn", bufs=4))
    outpool = ctx.enter_context(tc.tile_pool(name="os", bufs=OS_BUFS))

    # ---- persistent SBUF tensors ----
    # plane dim: 0 = h (fp8 high), 1 = l (fp8 residual)
    xhl_sb = persist.tile([P, KC, 2, T], FP8, tag="xhl")
    whl_sb = persist.tile([P, KC, 2, WCOLS], FP8, tag="whl")
    wphl_sb = persist.tile([P, 4, 2, C], FP8, tag="wphl")
    q_sb = persist.tile([P, 4 * T], BF16, tag="q")
    kT_sb = persist.tile([P, T], BF16, tag="k")
    v_sb = persist.tile([P, NT * 130], BF16, tag="v")
    yhl_sb = persist.tile([P, 2, 4, T], FP8, tag="yhl")
    bq_sb = persist.tile([P, 4], F32, tag="bq")
    bk_sb = persist.tile([P, 1], F32, tag="bk")
    ident = persist.tile([P, P], BF16, tag="ident")
    # triangle mask for diagonal blocks: trimask[x, y] = 1 if y >= x else 0
    trimask = persist.tile([P, P], BF16, tag="trimask")

    # ---- input DMAs ----
    # One DMA per (k-tile [, token chunk]): every DMA serializes ~630 ns on
    # the shared HWDGE device, so fewer/bigger transfers pace the startup.
    # Emission order matches consumption: w, x chunk 0, chunk 1, wp (needed
    # by cproj(0) during window 1), chunks 2-3.
    def wsrc(kq, pl):
        # 4 k-tiles of one w plane: [128, 4, WCOLS]
        return whl_d.ap()[4 * kq * P:(4 * kq + 4) * P, :].rearrange(
            "(kk p) (two w) -> p kk two w", p=P, two=2)[:, :, pl, :]

    def xsrc(kq, pl, n):
        return xhl_d.ap()[4 * kq * P:(4 * kq + 4) * P, :].rearrange(
            "(kk p) (two t) -> p kk two t", p=P, two=2)[:, :, pl, n * TQ:(n + 1) * TQ]

    def wdst(kq, pl):
        return whl_sb[:, 4 * kq:4 * kq + 4, pl, :]

    def xdst(kq, pl, n):
        return xhl_sb[:, 4 * kq:4 * kq + 4, pl, n * TQ:(n + 1) * TQ]

    # order matches chain consumption: (wh,xh) mains, then xl, then wl
    for kq in range(4):
        nc.sync.dma_start(wdst(kq, 0), wsrc(kq, 0))
        nc.sync.dma_start(xdst(kq, 0, 0), xsrc(kq, 0, 0))
    nc.sync.dma_start(bq_sb[:], bq_d.ap().rearrange("t p -> p t"))
    nc.sync.dma_start(bk_sb[:], bk_d.ap().rearrange("t p -> p t"))
    for kq in range(4):
        nc.sync.dma_start(xdst(kq, 1, 0), xsrc(kq, 1, 0))
    for kq in range(4):
        nc.sync.dma_start(wdst(kq, 1), wsrc(kq, 1))
    for kq in range(4):
        nc.sync.dma_start(xdst(kq, 0, 1), xsrc(kq, 0, 1))
        nc.sync.dma_start(xdst(kq, 1, 1), xsrc(kq, 1, 1))
    for k in range(4):
        nc.sync.dma_start(wphl_sb[:, k, :, :],
                          wphl_d.ap()[k * P:(k + 1) * P, :].rearrange(
                              "p (two c) -> p two c", two=2))
    for n in range(2, NJ):
        for kq in range(4):
            nc.sync.dma_start(xdst(kq, 0, n), xsrc(kq, 0, n))
            nc.sync.dma_start(xdst(kq, 1, n), xsrc(kq, 1, n))

    # ---- constants ----
    make_identity(nc, ident[:])
    mkf = ynpool.tile([P, P], F32, tag="yn", name="mkf")
    nc.gpsimd.memset(mkf[:], 1.0)
    nc.gpsimd.affine_select(
        out=mkf[:], in_=mkf[:], compare_op=mybir.AluOpType.is_ge,
        fill=0.0, base=0, pattern=[[1, P]], channel_multiplier=-1)
    nc.scalar.copy(trimask[:], mkf[:])
    nc.vector.memset(v_sb[:], 1.0)  # ones columns; data cols overwritten below
    # pre-zero the score psum tiles: exp may read never-written columns in
    # diagonal windows (they are masked later), which must be finite
    for w in range(2):
        pwarm = pps.tile([P, 2 * TQ], F32, tag="ps", name="pswarm")
        nc.vector.memset(pwarm[:], 0.0)

    # ---- qkv projection units (fp8 h+l DoubleRow) ----
    def qkv_matmuls(ps_ap, wcol0, wcols, tcol0, tcols, swap=False):
        """Three DoubleRow chains -- (wh,xh), (wh,xl), (wl,xh) -- each
        contracting 2 k-tiles per instruction.  swap: x stationary."""
        for ci, (wp_, xp_) in enumerate([(0, 0), (0, 1), (1, 0)]):
            for kp in range(KC // 2):
                wap = whl_sb[:, 2 * kp:2 * kp + 2, wp_, wcol0:wcol0 + wcols]
                xap = xhl_sb[:, 2 * kp:2 * kp + 2, xp_, tcol0:tcol0 + tcols]
                lhsT, rhs = (xap, wap) if swap else (wap, xap)
                nc.tensor.matmul(ps_ap, lhsT, rhs,
                                 start=(ci == 0 and kp == 0),
                                 stop=(ci == 2 and kp == KC // 2 - 1),
                                 perf_mode=DR)

    def unit_q(n, mt):
        def go():
            ps = ppm.tile([P, TQ], F32, tag="pm", name="psq")
            qkv_matmuls(ps[:], mt * P, P, n * TQ, TQ)
            nc.vector.tensor_scalar(
                out=q_sb[:, mt * T + n * TQ: mt * T + (n + 1) * TQ],
                in0=ps[:], scalar1=1.0 / WSCALE, scalar2=bq_sb[:, mt:mt + 1],
                op0=mult, op1=add)
        return go

    def unit_k(n):
        def go():
            ps = ppm.tile([P, TQ], F32, tag="pm", name="psk")
            qkv_matmuls(ps[:], QROWS, P, n * TQ, TQ)
            nc.vector.tensor_scalar(
                out=kT_sb[:, n * TQ:(n + 1) * TQ],
                in0=ps[:], scalar1=0.125 / WSCALE, scalar2=bk_sb[:, 0:1],
                op0=mult, op1=add)
        return go

    def unit_v(i):
        # v_sb tile i: [0:64]=kv0, 64=ones, [65:129]=kv1, 129=ones
        def go():
            ps = ppm.tile([P, TQ], F32, tag="pm", name="psv")
            qkv_matmuls(ps[:, 0:P], QROWS + P, P, i * P, P, swap=True)
            nc.vector.tensor_scalar(
                out=v_sb[:, i * 130: i * 130 + 64], in0=ps[:, 0:64],
                scalar1=1.0 / WSCALE, scalar2=None, op0=mult)
            nc.vector.tensor_scalar(
                out=v_sb[:, i * 130 + 65: i * 130 + 129], in0=ps[:, 64:128],
                scalar1=1.0 / WSCALE, scalar2=None, op0=mult)
        return go

    # ---- attention units ----
    def unit_attn(j, hp):
        # head pair (hp, hp+4): q/y column tile `hp`, head A on partitions
        # 0:64 of q/kT (kv0), head B on 64:128 (kv1).
        # Main generator: one yield per i-tile "slot"; PV trails scores by
        # 2 i-tiles so each PV matmul's exp finished a full slot earlier
        # (the PE is in-order -- a waiting matmul blocks the queue).
        # The normalize/transpose tail is returned as a separate generator
        # that the scheduler interleaves with the NEXT unit's slots.
        nb = 4 * (j + 1)   # tk tiles in play (block-causal)
        qcol = hp * T + j * TQ
        pv = [None, None]
        pv_first = [True, True]
        pts = {}
        ys = [None, None]

        def emit_scores_exp(i):
            r = i - 4 * j
            c0 = max(0, r) * P
            ps = pps.tile([P, 2 * TQ], F32, tag="ps", name="pss")
            for h in (0, 1):
                rb = 64 * h
                nc.tensor.matmul(
                    ps[:, TQ * h + c0: TQ * (h + 1)],
                    kT_sb[rb:rb + 64, i * P:(i + 1) * P],
                    q_sb[rb:rb + 64, qcol + c0: qcol + TQ],
                    start=True, stop=True)
            pt = ptpool.tile([P, 2 * TQ], BF16, tag="pt", name="pt")
            pts[i] = pt
            if c0 >= 2 * P:
                # skip the head-B hole [TQ, TQ+c0) -- two exps are cheaper
                # than the wasted columns once c0 >= 256
                nc.scalar.activation(pt[:, c0:TQ], ps[:, c0:TQ], ExpF)
                nc.scalar.activation(pt[:, TQ + c0:2 * TQ], ps[:, TQ + c0:2 * TQ], ExpF)
            else:
                nc.scalar.activation(pt[:, c0:2 * TQ], ps[:, c0:2 * TQ], ExpF)
            if r >= 0:
                for h in (0, 1):
                    tri = pt[:, TQ * h + c0: TQ * h + c0 + P]
                    MASK_ENG.tensor_tensor(out=tri, in0=tri, in1=trimask[:], op=mult)

        def emit_pv(i):
            pt = pts.pop(i)
            r = i - 4 * j
            for h in (0, 1):
                for s in range(max(0, r), 4):
                    first = pv_first[h]
                    pv_first[h] = False
                    nc.tensor.matmul(
                        pv[h][:, 65 * s: 65 * s + 65],
                        pt[:, TQ * h + P * s: TQ * h + P * (s + 1)],
                        v_sb[:, i * 130 + 65 * h: i * 130 + 65 * h + 65],
                        start=first, stop=(i == nb - 1 and s == 3),
                        skip_group_check=not first)

        def main_gen():
            pv[0] = ppv.tile([P, 4 * 65], F32, tag="pv", name="pvA")
            pv[1] = ppv.tile([P, 4 * 65], F32, tag="pv", name="pvB")
            for i in range(nb):
                emit_scores_exp(i)
                if i >= PV_LAG:
                    emit_pv(i - PV_LAG)
                yield 650
            if j in INLINE_TRAIL_W:
                # emit trailing PVs + drains inline: the next unit's
                # scores/exp reach the (ACT-bound) pipeline sooner
                for i in range(max(0, nb - PV_LAG), nb):
                    emit_pv(i)
                for h in (0, 1):
                    ys[h] = yspool.tile([P, 4 * 65], F32, tag="ys", name=f"ys{h}")
                    nc.vector.tensor_copy(ys[h][:], pv[h][:])
                yield 650
            else:
                for i in range(max(0, nb - PV_LAG), nb - 1):
                    emit_pv(i)
                    yield 350
                emit_pv(nb - 1)
                # drain the PV psum banks so the next unit can claim them
                for h in (0, 1):
                    ys[h] = yspool.tile([P, 4 * 65], F32, tag="ys", name=f"ys{h}")
                    nc.vector.tensor_copy(ys[h][:], pv[h][:])
                yield 350

        def tail_gen():
            # normalize into bf16 (Pool), transpose in bf16 (PE), then h/l
            # fp8 split during the psum->sbuf copies (DVE).  fp8 values are
            # exact in bf16, so yh+yl reproduces the bf16 y exactly.
            for s in range(4):
                for h in (0, 1):
                    o = P * s + 64 * h + hp * TQ
                    nc.gpsimd.normalize_recip(
                        ynf[:, o: o + 64],
                        ys[h][:, 65 * s: 65 * s + 64],
                        ys[h][:, 65 * s + 64: 65 * s + 65])
                yield 0
            yield 0
            ptr = ppm.tile([P, TQ], BF16, tag="pm", name="ptr")
            for s in range(4):
                nc.tensor.matmul(
                    ptr[:, P * s: P * (s + 1)],
                    ynf[:, P * s + hp * TQ: P * (s + 1) + hp * TQ],
                    ident[:], is_transpose=True,
                    start=(s == 0), stop=(s == 3), skip_group_check=(s != 0))
            yh_dst = yhl_sb[:, 0, hp, j * TQ: (j + 1) * TQ]
            nc.vector.tensor_copy(yh_dst, ptr[:])
            nc.vector.tensor_tensor(
                out=yhl_sb[:, 1, hp, j * TQ: (j + 1) * TQ],
                in0=ptr[:], in1=yh_dst, op=mybir.AluOpType.subtract)
        return main_gen(), tail_gen

    def unit_cproj(j, ms, split_dma=False):
        def gen():
            os_t = outpool.tile([P, 4 * TQ], F32, tag="os", name="os")
            for n in range(NJ):
                pc = ppm.tile([P, TQ], F32, tag="pm", name="pc")
                for ii, (kp, (yp, wpp)) in enumerate(
                        [(kp, c) for kp in (0, 1)
                         for c in ((0, 0), (0, 1), (1, 0))]):
                    nc.tensor.matmul(
                        pc[:],
                        yhl_sb[:, yp, 2 * kp:2 * kp + 2,
                               j * TQ + ms * P: j * TQ + (ms + 1) * P],
                        wphl_sb[:, 2 * kp:2 * kp + 2, wpp,
                                n * TQ:(n + 1) * TQ],
                        start=(ii == 0), stop=(ii == 5), perf_mode=DR)
                    if ii == 2:
                        yield 330
                nc.vector.tensor_scalar(
                    out=os_t[:, n * TQ:(n + 1) * TQ], in0=pc[:],
                    scalar1=1.0 / WSCALE, scalar2=None, op0=mult)
                if split_dma:
                    nc.sync.dma_start(
                        out_d.ap()[j * TQ + ms * P: j * TQ + (ms + 1) * P,
                                   n * TQ:(n + 1) * TQ],
                        os_t[:, n * TQ:(n + 1) * TQ])
                yield 430
            if not split_dma:
                nc.sync.dma_start(
                    out_d.ap()[j * TQ + ms * P: j * TQ + (ms + 1) * P, :], os_t[:])
        return gen()

    def gen_q(n, mt):
        def gen():
            ps = ppm.tile([P, TQ], F32, tag="pm", name="psq")
            for seg in _qkv_segs(ps[:], mt * P, P, n * TQ, TQ):
                yield seg
            nc.vector.tensor_scalar(
                out=q_sb[:, mt * T + n * TQ: mt * T + (n + 1) * TQ],
                in0=ps[:], scalar1=1.0 / WSCALE, scalar2=bq_sb[:, mt:mt + 1],
                op0=mult, op1=add)
        return gen()

    def gen_k(n):
        def gen():
            ps = ppm.tile([P, TQ], F32, tag="pm", name="psk")
            for seg in _qkv_segs(ps[:], QROWS, P, n * TQ, TQ):
                yield seg
            nc.vector.tensor_scalar(
                out=kT_sb[:, n * TQ:(n + 1) * TQ],
                in0=ps[:], scalar1=0.125 / WSCALE, scalar2=bk_sb[:, 0:1],
                op0=mult, op1=add)
        return gen()

    def _qkv_segs(ps_ap, wcol0, wcols, tcol0, tcols, swap=False):
        emitted = 0
        for ci, (wp_, xp_) in enumerate([(0, 0), (0, 1), (1, 0)]):
            for kp in range(KC // 2):
                wap = whl_sb[:, 2 * kp:2 * kp + 2, wp_, wcol0:wcol0 + wcols]
                xap = xhl_sb[:, 2 * kp:2 * kp + 2, xp_, tcol0:tcol0 + tcols]
                lhsT, rhs = (xap, wap) if swap else (wap, xap)
                nc.tensor.matmul(ps_ap, lhsT, rhs,
                                 start=(ci == 0 and kp == 0),
                                 stop=(ci == 2 and kp == KC // 2 - 1),
                                 perf_mode=DR)
                emitted += 1
                if emitted % 6 == 0 and emitted < 24:
                    yield 640

    def gen_v(i):
        # v_sb tile i: [0:64]=kv0, 64=ones, [65:129]=kv1, 129=ones
        def gen():
            ps = ppm.tile([P, TQ], F32, tag="pm", name="psv")
            for seg in _qkv_segs(ps[:, 0:P], QROWS + P, P, i * P, P, swap=True):
                pass  # 27 ns per matmul; no need to split
            nc.vector.tensor_scalar(
                out=v_sb[:, i * 130: i * 130 + 64], in0=ps[:, 0:64],
                scalar1=1.0 / WSCALE, scalar2=None, op0=mult)
            nc.vector.tensor_scalar(
                out=v_sb[:, i * 130 + 65: i * 130 + 129], in0=ps[:, 64:128],
                scalar1=1.0 / WSCALE, scalar2=None, op0=mult)
            yield 650
        return gen()

    def proj_gens(n):
        return ([gen_q(n, mt) for mt in range(4)] + [gen_k(n)]
                + [gen_v(i) for i in range(4 * n, 4 * n + 4)])

    # yn staging buffers for normalize->transpose, one [P, TQ] region per pair
    ynf = persist.tile([P, 4 * TQ], BF16, tag="ynf")

    class FillerQueue:
        """Sequential queue of generator units; pulls ~budget ns of PE
        segments at a time."""

        def __init__(self):
            self.gens = []
            self.cur = None
            self.done = 0

        def add(self, gens):
            self.gens.extend(gens)

        def push_front(self, gen):
            self.gens.insert(0, gen)

        def pull(self, budget):
            got = 0
            while got < budget:
                if self.cur is None:
                    if not self.gens:
                        return got
                    self.cur = self.gens.pop(0)
                try:
                    got += next(self.cur)
                except StopIteration:
                    self.cur = None
                    self.done += 1
            return got

        def drain(self):
            while self.pull(1 << 30) > 0:
                pass

    # ---- software-pipelined emission ----
    # Three filler queues by priority: fqd (this window's deferred q m-tiles
    # 1-3 -- needed by attn units 1-3 of the SAME window), fqp (next window's
    # critical projections: q m-tile 0, k, v), fqf (cproj -- fully flexible,
    # carried across windows to feed the ACT-bound late windows).
    fqd = FillerQueue()
    fqp = FillerQueue()
    fqf = FillerQueue()

    def pull(budget):
        got = fqd.pull(budget)
        if got < budget:
            got += fqp.pull(budget - got)
        if got < budget:
            fqf.pull(budget - got)

    def step(g):
        if g is None:
            return None
        try:
            next(g)
            return g
        except StopIteration:
            return None

    # prologue: everything attn(0, hp=0) touches must be fully emitted
    # before its PV reads are emitted (emission order = dependency order)
    fqp.add([gen_q(0, 0), gen_k(0)] + [gen_v(i) for i in range(4)])
    fqp.drain()
    pending_tail = None
    deferred_q = {j: [gen_q(j, mt) for mt in (1, 2, 3)] for j in range(NJ)}
    for j in range(NJ):
        fqd.add(deferred_q[j])
        if j + 1 < NJ:
            fqp.add([gen_q(j + 1, 0), gen_k(j + 1)]
                    + [gen_v(i) for i in range(4 * (j + 1), 4 * (j + 1) + 4)])
        nslots = 4 * (4 * (j + 1) + 3)
        budget = BUDGETS[j] / nslots
        done0 = fqd.done
        for hp in range(4):
            if hp >= 1:
                # attn(j, hp) needs q(j, hp): force any unemitted remainder
                while (fqd.done - done0 < hp
                       and (fqd.cur is not None or fqd.gens)):
                    fqd.pull(700)
            main, tail = unit_attn(j, hp)
            for si, hint in enumerate(main):
                pull(budget)
                for _ in range(TAIL_STEPS):
                    pending_tail = step(pending_tail)
                if hp == 0 and si == 6:
                    # attn(j-1, 3)'s tail has fully emitted its yhl write
                    # (guaranteed by the flush below); cproj may follow it
                    for (jj, ms) in CPROJ_AT.get(j, []):
                        fqf.add([unit_cproj(jj, ms)])
            # never drop an unfinished tail: its yhl copy must be emitted
            while pending_tail is not None:
                pending_tail = step(pending_tail)
            pending_tail = tail()
        fqp.drain()
    while pending_tail is not None:
        pending_tail = step(pending_tail)
        pull(500)
    fqf.add([unit_cproj(3, ms, split_dma=True) for ms in range(4)])
    fqf.drain()


def _prep_inputs(x, w_attn, b_attn, w_proj):
    """Host-side shard + transpose + fp8 h+l split for each of the 8 cores."""
    in_maps = []
    xhl = {}
    for b in range(B):
        xT = np.ascontiguousarray(np.asarray(x[b], np.float32).T)
        xh = xT.astype(NPFP8)
        xl = (xT - xh.astype(np.float32)).astype(NPFP8)
        xhl[b] = np.concatenate([xh, xl], axis=1)  # [C, 2T]
    for g in range(N_CORES):
        b, grp = divmod(g, 4)

        q_rows = []
        for lh in Q_ORDER:
            gh = HL * grp + lh
            q_rows.extend(range(HS * gh, HS * gh + HS))
        k0 = NE + KROWS * grp
        v0 = NE + N_KV * HS + KROWS * grp
        rows = q_rows + list(range(k0, k0 + KROWS)) + list(range(v0, v0 + KROWS))
        wqkvT = np.ascontiguousarray(w_attn[rows, :].T) * WSCALE
        wh = wqkvT.astype(NPFP8)
        wl = (wqkvT - wh.astype(np.float32)).astype(NPFP8)
        whl = np.concatenate([wh, wl], axis=1)  # [C, 2*WCOLS]

        cols = []
        for lh in Q_ORDER:
            gh = HL * grp + lh
            cols.extend(range(HS * gh, HS * gh + HS))
        wpT = np.ascontiguousarray(w_proj[:, cols].T) * WSCALE
        wph = wpT.astype(NPFP8)
        wpl = (wpT - wph.astype(np.float32)).astype(NPFP8)
        wphl = np.concatenate([wph, wpl], axis=1)  # [QROWS, 2C]

        bq = np.asarray(b_attn[q_rows], np.float32).reshape(4, P)
        bk = (np.asarray(b_attn[k0:k0 + KROWS], np.float32) / 8.0).reshape(1, P)

        in_maps.append({"xhl": xhl[b], "whl": whl, "wphl": wphl,
                        "bq": bq, "bk": bk})
    return in_maps


def get_nc():
    if "nc" not in _CACHE:
        _CACHE["nc"] = _build_program()
    return _CACHE["nc"]


def kernel(x, w_attn, b_attn, w_proj, b_proj):
    x = np.asarray(x, np.float32)
    w_attn = np.asarray(w_attn, np.float32)
    b_attn = np.asarray(b_attn, np.float32)
    w_proj = np.asarray(w_proj, np.float32)
    b_proj = np.asarray(b_proj, np.float32)

    nc = get_nc()
    in_maps = _prep_inputs(x, w_attn, b_attn, w_proj)
    res = run_bass_kernel_spmd(nc, in_maps, core_ids=list(range(N_CORES)))

    # host "all-reduce" over the 4 head-group cores per batch + bias folds
    bv = b_attn[NE + N_KV * HS:]                      # [512] v bias
    bv_full = np.repeat(bv.reshape(N_KV, HS), N_HEAD // N_KV, axis=0).reshape(-1)
    delta = bv_full @ w_proj.T + b_proj               # [2048]
    out = np.zeros((B, T, C), np.float32)
    for g in range(N_CORES):
        b = g // 4
        out[b] += res.results[g]["out"]
    out += delta[None, None, :]
    return out
